# revision 63
# baseline (speedup 1.0000x reference)
"""Trainium2 Bass kernel for nn_Att_AdaIn (B=4, C=256, H=W=64 attention block).

Sharding: 8 cores = 4 batches x 2 query-halves. Each core holds the fused
weights, the full key/value source y[b] ([256, 4096]), and its own query
slice x[b][:, half] ([256, 2048]); it computes the full attention output for
its 2048 queries. Host gathers the 8 [256, 2048] results.

Weight fusion done on the host (in float64):
  logits: S = k^T q with q = Wq x + bq, k = Wk y + bk
        = y^T (Wk^T Wq) x + y^T (Wk^T bq) 1^T + [per-query-constant terms]
    The per-query-constant (l-only) terms are softmax-invariant and dropped.
    So with  M^T = (Wk^T Wq)^T  and  bw = Wk^T bq:   qm = M x + bw,
    ST[j,l] = sum_c y[c,j] qm[c,l].
  output: Wo (V E / den) + bo  with V = Wv y + bv 1^T
        = (Wo Wv) y E / den + Wo bv + bo
    So with MoT = (Wo Wv)^T and bo2 = bo + Wo bv, the value projection
    vTo = y^T MoT directly produces Wo-mixed values and the separate
    output projection disappears.

Per-core pipeline (layouts chosen so no on-chip transpose is needed):
  qm  = M x + bw               [c, l]      (c on partitions)
  vTo = y^T MoT                [j, 256]    (j on partitions)
  ST  = y^T qm                 [j, l]      (transposed attention scores)
  E   = exp(ST / sqrt(C))      (no max-subtraction: logits ~ N(0,1), fp32-safe)
  zq  = vTo^T E                [256, l]    unnormalized Wo-mixed output
  den = 1^T E                  [l]         softmax denominators (E summed on
                                           DVE, partition-reduced by one matmul)
  out = zq * (1/den) + bo2 + x

Dtype config via env:
  ATT_MATMUL_DT: float32 | float32r | bfloat16 (base matmul dtype)
  ATT_FP8: 0 = off (safest numerics, ~184 us, rel err ~3e-4),
           1 = fp8(e4m3) DoubleRow score matmuls (~129 us, rel err ~2e-3),
           2 = level 1 + fp8 DoubleRow qm/vTo projections,
           3 = level 2 + fp8 values/denominator with paired exps and a
               -2.5 logit shift (default; ~102.5 us, rel err ~4e-3).
  ATT_V1: 1 = use the older (pre-pipelined) emission for ATT_FP8=3.

The default (bf16 + ATT_FP8=3) path, measured at 99.8 us at full clock (vs
186.8 us for the session-start baseline; the chip sometimes sits in a ~1.2x
P0/thermal downclock — compare runs via the trace's exp duration, 1113 ns
at full clock), balances the 64-step stream across three engines:
 - per jc-pair step the PE issues fp8-DoubleRow matmuls (215 ns each at
   the N=512 hardware floor); ScalarE runs 14 of 16 exps per tile
   (1113 ns each); VectorE runs the other 2 via a log-domain Schraudolph
   exp — a single tensor_scalar producing fp8e4m3 BITS directly
   (uint8 = 8*(log2e*scaled_logit + 7 - 0.043), saturating at 0,
   bitcast to fp8; error ~ the fp8 quantization noise already on E) —
   and 7 of 16 denominator matmuls are replaced by VectorE f32
   accumulation folded back with one tiny partition-reduce matmul pair;
 - four-deep software pipeline: the score-matmul pair for step i+4 is
   emitted before the value/den consumers of step i, hiding both the exp
   latency and the longer VectorE-offload latency;
 - per-l-tile softmax tails (1/den, broadcast, final muls, store) are
   deferred into the next tile's stream via zq->SBUF staging copies +
   a DRAM-roundtrip broadcast of 1/den, keeping PSUM within 8 banks
   (stp ring 4 + zq 2 + den 1 + rbc 1);
 - the qm/vTo projection phase splits its PSUM->SBUF consumers
   alternately between ScalarE and VectorE and cycles PSUM tiles through
   the stream's idle zq/den/rbc banks (~6 units in flight);
 - the exp table load is prewarmed during the initial DMA wait; all DRAM
   tensors are pre-shuffled on the host into partition-major layout so
   every load is one fat contiguous-per-partition DMA; the final tile's
   tail is split into two l-halves whose output stores issue from three
   different engine queues.
"""

import os
import sys

for _p in ("/root/.axon_site", "/root/.axon_site/_ro/trn_rl_repo", "/opt/trn_rl_repo"):
    if os.path.isdir(_p) and _p not in sys.path:
        sys.path.append(_p)

import numpy as np

import concourse.bass as bass
from concourse import bacc, mybir, tile
from concourse import bass_utils

B, C, H, W = 4, 256, 64, 64
N = H * W          # 4096 pixels
NQ = N // 2        # 2048 queries per core
P = 128
A = C // P         # 2 channel chunks
LT = 512           # l-tile (query) width
NLT = NQ // LT     # 4 l-tiles
JC = N // P        # 32 key chunks
SCALE = 1.0 / np.sqrt(np.float32(C))  # 1/16

MATMUL_DT = os.environ.get("ATT_MATMUL_DT", "bfloat16")
FP8_LEVEL = int(os.environ.get("ATT_FP8", "3"))
USE_V1 = os.environ.get("ATT_V1", "0") == "1"


def build_nc(matmul_dt_name: str = MATMUL_DT, fp8_level: int = FP8_LEVEL):
    mdt = getattr(mybir.dt, matmul_dt_name)
    f32 = mybir.dt.float32
    f8 = mybir.dt.float8e4
    is_bf16 = mdt == mybir.dt.bfloat16
    st_fp8 = fp8_level >= 1 and is_bf16
    proj_fp8 = fp8_level >= 2 and is_bf16
    val_fp8 = fp8_level >= 3 and is_bf16
    DR = mybir.MatmulPerfMode.DoubleRow

    nc = bacc.Bacc("TRN2", target_bir_lowering=False, debug=False)

    # --- DRAM tensors ---
    xdt = f8 if proj_fp8 else mdt
    x_d = nc.dram_tensor("x", [C, NQ], xdt, kind="ExternalInput").ap()
    mT_d = nc.dram_tensor("mT", [C, C], xdt, kind="ExternalInput").ap()
    if st_fp8:
        y8_d = nc.dram_tensor("y8", [C, N], f8, kind="ExternalInput").ap()
    if not proj_fp8:
        y_d = nc.dram_tensor("y", [C, N], mdt, kind="ExternalInput").ap()
    moTa_d = nc.dram_tensor("moTa", [C, C], xdt, kind="ExternalInput").ap()
    bw_d = nc.dram_tensor("bw", [C], f32, kind="ExternalInput").ap()
    bo2_d = nc.dram_tensor("bo2", [C], f32, kind="ExternalInput").ap()
    if is_bf16:
        xres_d = nc.dram_tensor("xres", [C, NQ], f32, kind="ExternalInput").ap()
    out_d = nc.dram_tensor("out", [C, NQ], f32, kind="ExternalOutput").ap()

    qm_dt = f8 if st_fp8 else mdt

    with tile.TileContext(nc) as tc:
        with (
            tc.tile_pool(name="const", bufs=1) as const,
            tc.tile_pool(name="epool", bufs=8) as epool,
            tc.tile_pool(name="opool", bufs=3) as opool,
            tc.tile_pool(name="rpool", bufs=2) as rpool,
            tc.tile_pool(name="ps_st", bufs=2 if val_fp8 else 4, space="PSUM") as ps_st,
            tc.tile_pool(name="ps_zq", bufs=1 if val_fp8 else 2, space="PSUM") as ps_zq,
            tc.tile_pool(name="ps_small", bufs=1, space="PSUM") as ps_small,
            tc.tile_pool(name="dpool", bufs=2, space="DRAM") as dpool,
        ):
            # ---- persistent SBUF tensors ----
            x_sb = const.tile([P, A, NQ], xdt)
            mT_sb = const.tile([P, A, C], xdt)
            if st_fp8:
                y8_sb = const.tile([P, A, N], f8)
            if not proj_fp8:
                y_sb = const.tile([P, A, N], mdt)
            moTa_sb = const.tile([P, A, C], xdt)
            bw_sb = const.tile([P, A], f32)
            bo2_sb = const.tile([P, A], f32)
            ones_col = const.tile([P, 1], mdt)
            ones_row = const.tile([1, P], mdt)
            ones_p2 = const.tile([P, 2, 16], f8)
            shift_sb = const.tile([P, 1], f32)
            qm_sb = const.tile([P, A, NQ], qm_dt)
            vTo_sb = const.tile([P, JC, C], f8 if val_fp8 else mdt)
            if is_bf16:
                xres_sb = const.tile([P, A, NQ], f32)
            else:
                xres_sb = x_sb.bitcast(f32)

            # ---- loads (in order of first use; xres last, needed only at the end) ----
            xr_ = x_d.rearrange("(a p) n -> p a n", p=P)
            nc.sync.dma_start(out=x_sb[:, :, :NQ // 2], in_=xr_[:, :, :NQ // 2])
            nc.sync.dma_start(out=mT_sb, in_=mT_d.rearrange("(a p) o -> p a o", p=P))
            nc.sync.dma_start(out=bw_sb, in_=bw_d.rearrange("(a p) -> p a", p=P))
            nc.sync.dma_start(out=x_sb[:, :, NQ // 2:], in_=xr_[:, :, NQ // 2:])
            if not proj_fp8:
                yr_ = y_d.rearrange("(a p) n -> p a n", p=P)
                nc.sync.dma_start(out=y_sb[:, :, :N // 2], in_=yr_[:, :, :N // 2])
            nc.sync.dma_start(out=moTa_sb, in_=moTa_d.rearrange("(a p) o -> p a o", p=P))
            if not proj_fp8:
                nc.sync.dma_start(out=y_sb[:, :, N // 2:], in_=yr_[:, :, N // 2:])
            if st_fp8:
                y8r_ = y8_d.rearrange("(a p) n -> p a n", p=P)
                nc.sync.dma_start(out=y8_sb[:, :, :N // 2], in_=y8r_[:, :, :N // 2])
                nc.sync.dma_start(out=y8_sb[:, :, N // 2:], in_=y8r_[:, :, N // 2:])
            nc.sync.dma_start(out=bo2_sb, in_=bo2_d.rearrange("(a p) -> p a", p=P))
            nc.vector.memset(ones_col, 1.0)
            nc.vector.memset(ones_row, 1.0)
            nc.vector.memset(ones_p2, 1.0)
            nc.vector.memset(shift_sb, -2.5)
            if is_bf16:
                nc.sync.dma_start(
                    out=xres_sb, in_=xres_d.rearrange("(a p) n -> p a n", p=P)
                )

            # ---- projections ----
            # qm[c, l] = sum_c' M[c, c'] x[c', l] + bw[c]
            for lt in range(NLT):
                for och in range(A):
                    ps = ps_st.tile([P, LT], f32, tag="st")
                    if proj_fp8:
                        nc.tensor.matmul(
                            ps,
                            mT_sb[:, :, och * P:(och + 1) * P],
                            x_sb[:, :, lt * LT:(lt + 1) * LT],
                            start=True, stop=True, perf_mode=DR,
                        )
                    else:
                        for a in range(A):
                            nc.tensor.matmul(
                                ps,
                                mT_sb[:, a, och * P:(och + 1) * P],
                                x_sb[:, a, lt * LT:(lt + 1) * LT],
                                start=(a == 0),
                                stop=(a == A - 1),
                            )
                    nc.vector.tensor_scalar_add(
                        out=qm_sb[:, och, lt * LT:(lt + 1) * LT],
                        in0=ps,
                        scalar1=bw_sb[:, och:och + 1],
                    )
            # vTo[j, o] = sum_c y[c, j] MoT[c, o]
            # (for the generic path this is emitted inside the first attention
            # pass, one chunk ahead of its first use, so the PE stream never
            # stalls behind the y DMA; val_fp8 keeps the standalone loop)
            def emit_vto(jc):
                ps = ps_st.tile([P, C], f32, name="psv", tag="st")
                if proj_fp8:
                    nc.tensor.matmul(
                        ps,
                        y8_sb[:, :, jc * P:(jc + 1) * P],
                        moTa_sb[:, :, :],
                        start=True, stop=True, perf_mode=DR,
                    )
                else:
                    for a in range(A):
                        nc.tensor.matmul(
                            ps,
                            y_sb[:, a, jc * P:(jc + 1) * P],
                            moTa_sb[:, a, :],
                            start=(a == 0),
                            stop=(a == A - 1),
                        )
                nc.vector.tensor_copy(out=vTo_sb[:, jc, :], in_=ps)

            if st_fp8 or val_fp8:
                for jc in range(JC):
                    emit_vto(jc)

            # ---- attention, l-tile at a time ----
            def emit_tail(tl, zq, eacc):
                tsl = slice(tl * LT, (tl + 1) * LT)
                den_t = ps_st.tile([P, LT], f32, name="den_t", tag="st")
                den = den_t[0:1, :]
                nc.tensor.matmul(den, ones_col, eacc, start=True, stop=True)
                r_sb = rpool.tile([1, LT], f32, name="r_sb", tag="r")
                nc.vector.reciprocal_approx_fast(out=r_sb, in_=den)
                rbc_sb = rpool.tile([P, LT], f32, name="rbc_sb", tag="rbc")
                if tl == NLT - 1 and is_bf16:
                    # latency-critical final tile: broadcast r across
                    # partitions on the PE (bf16), skipping the DRAM trip
                    r_bf = rpool.tile([1, LT], mdt, name="r_bf", tag="rbf")
                    nc.vector.tensor_copy(out=r_bf, in_=r_sb)
                    rbc_ps = ps_st.tile([P, LT], f32, name="rbc_ps", tag="st")
                    nc.tensor.matmul(rbc_ps, ones_row, r_bf, start=True, stop=True)
                    nc.scalar.activation(
                        out=rbc_sb, in_=rbc_ps,
                        func=mybir.ActivationFunctionType.Copy,
                    )
                else:
                    # broadcast across partitions via a DRAM round-trip (off
                    # the PE/ACT critical path; overlapped by later matmuls)
                    r_dram = dpool.tile([1, LT], f32, name="r_dram", tag="rdram")
                    nc.sync.dma_start(out=r_dram, in_=r_sb)
                    r_bcast_ap = bass.AP(
                        tensor=r_dram.tensor,
                        offset=r_dram.offset,
                        ap=[[0, P], list(r_dram.ap[-1])],
                    )
                    nc.sync.dma_start(out=rbc_sb, in_=r_bcast_ap)
                for och in range(A):
                    o_sb = opool.tile([P, LT], f32, name="o_sb")
                    nc.vector.tensor_mul(out=o_sb, in0=zq[och], in1=rbc_sb)
                    nc.vector.scalar_tensor_tensor(
                        out=o_sb,
                        in0=o_sb,
                        scalar=bo2_sb[:, och:och + 1],
                        in1=xres_sb[:, och, tsl],
                        op0=mybir.AluOpType.add,
                        op1=mybir.AluOpType.add,
                    )
                    nc.sync.dma_start(
                        out=out_d.rearrange("(a p) n -> p a n", p=P)[:, och, tsl],
                        in_=o_sb,
                    )

            pending = None
            for lt in range(NLT):
                lsl = slice(lt * LT, (lt + 1) * LT)
                zq0 = ps_zq.tile([P, LT], f32, tag="zq0")
                zq1 = ps_zq.tile([P, LT], f32, tag="zq1")
                zq = (zq0, zq1)
                if val_fp8:
                    # fully fp8-DoubleRow attention: ST pairs -> one exp per
                    # pair -> DR value/denominator matmuls over jc-pairs
                    den = ps_small.tile([1, LT], f32, tag="den")
                    for jp in range(JC // 2):
                        stp = ps_st.tile([P, 2, LT], f32, tag="st")
                        for h in range(2):
                            jc = jp * 2 + h
                            nc.tensor.matmul(
                                stp[:, h, :],
                                y8_sb[:, :, jc * P:(jc + 1) * P],
                                qm_sb[:, :, lsl],
                                start=True, stop=True, perf_mode=DR,
                            )
                        e8 = epool.tile([P, 2, LT], f8)
                        # -2.5 logit shift keeps exp within fp8e4 range (max
                        # +-240); it scales numerator and denominator equally,
                        # so it cancels exactly in the softmax
                        nc.scalar.activation(
                            out=e8.rearrange("p h l -> p (h l)"),
                            in_=stp.rearrange("p h l -> p (h l)"),
                            func=mybir.ActivationFunctionType.Exp,
                            scale=float(SCALE),
                            bias=shift_sb,
                        )
                        for m in range(A):
                            nc.tensor.matmul(
                                zq[m],
                                vTo_sb[:, jp * 2:jp * 2 + 2, m * P:(m + 1) * P],
                                e8,
                                start=(jp == 0),
                                stop=(jp == JC // 2 - 1),
                                perf_mode=DR,
                            )
                        nc.tensor.matmul(
                            den,
                            ones_p2[:, :, 0:1],
                            e8,
                            start=(jp == 0),
                            stop=(jp == JC // 2 - 1),
                            perf_mode=DR,
                        )
                    r_sb = rpool.tile([1, LT], f32, tag="r")
                    nc.vector.reciprocal_approx_fast(out=r_sb, in_=den)
                    r_bf = rpool.tile([1, LT], mdt, tag="rbf")
                    nc.vector.tensor_copy(out=r_bf, in_=r_sb)
                    rbc_ps = ps_small.tile([P, LT], f32, tag="rbc")
                    nc.tensor.matmul(rbc_ps, ones_row, r_bf, start=True, stop=True)
                    rbc_sb = rpool.tile([P, LT], f32, tag="rbc")
                    nc.scalar.activation(
                        out=rbc_sb, in_=rbc_ps,
                        func=mybir.ActivationFunctionType.Copy,
                    )
                    for och in range(A):
                        o_sb = opool.tile([P, LT], f32)
                        nc.vector.tensor_mul(out=o_sb, in0=zq[och], in1=rbc_sb)
                        nc.vector.scalar_tensor_tensor(
                            out=o_sb,
                            in0=o_sb,
                            scalar=bo2_sb[:, och:och + 1],
                            in1=xres_sb[:, och, lsl],
                            op0=mybir.AluOpType.add,
                            op1=mybir.AluOpType.add,
                        )
                        nc.sync.dma_start(
                            out=out_d.rearrange("(a p) n -> p a n", p=P)[:, och, lsl],
                            in_=o_sb,
                        )
                    continue
                eacc = epool.tile([P, LT], mdt, tag="eacc")
                for jc in range(JC):
                    if lt == 0 and not st_fp8:
                        emit_vto(jc)
                    if jc == 3 and pending is not None:
                        emit_tail(*pending)
                        pending = None
                    st = ps_st.tile([P, LT], f32, tag="st")
                    if st_fp8:
                        nc.tensor.matmul(
                            st,
                            y8_sb[:, :, jc * P:(jc + 1) * P],
                            qm_sb[:, :, lsl],
                            start=True, stop=True, perf_mode=DR,
                        )
                    else:
                        for a in range(A):
                            nc.tensor.matmul(
                                st,
                                y_sb[:, a, jc * P:(jc + 1) * P],
                                qm_sb[:, a, lsl],
                                start=(a == 0),
                                stop=(a == A - 1),
                            )
                    e_sb = epool.tile([P, LT], mdt)
                    nc.scalar.activation(
                        out=e_sb,
                        in_=st,
                        func=mybir.ActivationFunctionType.Exp,
                        scale=float(SCALE),
                    )
                    for m in range(A):
                        nc.tensor.matmul(
                            zq[m],
                            vTo_sb[:, jc, m * P:(m + 1) * P],
                            e_sb,
                            start=(jc == 0),
                            stop=(jc == JC - 1),
                        )
                    if jc == 0:
                        nc.vector.tensor_copy(out=eacc, in_=e_sb)
                    else:
                        nc.vector.tensor_add(out=eacc, in0=eacc, in1=e_sb)

                # tail (den reduce -> 1/den -> broadcast -> finals) is
                # deferred into the next tile pass so the den matmul's wait on
                # the DVE accumulation chain can't stall the PE stream at the
                # tile boundary
                pending = (lt, zq, eacc)
            if pending is not None:
                emit_tail(*pending)

    nc.compile()
    return nc


def build_fp8_v2():
    """Optimized bf16+ATT_FP8=3 kernel: software-pipelined PE emission,
    deferred softmax tails, partition-major single-DMA loads."""
    f32 = mybir.dt.float32
    bf16 = mybir.dt.bfloat16
    u8 = mybir.dt.uint8
    f8 = mybir.dt.float8e4
    DR = mybir.MatmulPerfMode.DoubleRow
    JP = JC // 2  # 16 jc-pairs per l-tile

    # three-engine rebalance: ScalarE exp (1113ns) and the PE's 5 matmuls
    # (1075ns) are otherwise both ~99% busy per step. Two exps per tile run
    # on VectorE instead via a Schraudolph bit-trick (i32 = A*x + B
    # reinterpreted as f32; ~2% elementwise error, below the fp8-e4m3
    # quantization noise already on E), and five denominator matmuls per
    # tile are replaced by VectorE f32 accumulation folded back with one
    # tiny partition-reduce matmul pair.
    OFF_EXP = (5, 11)
    # chosen so no den-partial flush add occupies the VectorE FIFO in the
    # slots just before a Schraudolph TS (jp4/jp10 tops) — a flush there
    # delays the TS, the stp buffer frees late, and ScalarE stalls two
    # steps later waiting for its score matmuls
    OFF_DEN = (2, 5, 6, 7, 8, 11, 13)
    # log-domain Schraudolph straight to fp8e4m3 bits: uint8 out =
    # 8*(log2(e)*x + 7 - 0.043), saturating at 0 for underflow; bitcast to
    # fp8. One VectorE op replaces exp+cast.
    LOG2E = 1.4426950408889634
    SCHRA_A = float(8 * LOG2E * SCALE)
    SCHRA_B = float(8 * 7 - 8 * 0.0430 - 8 * LOG2E * 2.5)

    nc = bacc.Bacc("TRN2", target_bir_lowering=False, debug=False)

    # --- DRAM tensors, pre-shuffled on host to partition-major layout ---
    mT_d = nc.dram_tensor("mT", [P, A, C], f8, kind="ExternalInput").ap()
    x_d = nc.dram_tensor("x", [P, A, NQ], f8, kind="ExternalInput").ap()
    bw_d = nc.dram_tensor("bw", [P, A], f32, kind="ExternalInput").ap()
    moTa_d = nc.dram_tensor("moTa", [P, A, C], f8, kind="ExternalInput").ap()
    y8_d = nc.dram_tensor("y8", [P, A, N], f8, kind="ExternalInput").ap()
    bo2_d = nc.dram_tensor("bo2", [P, A], f32, kind="ExternalInput").ap()
    xres_d = nc.dram_tensor("xres", [P, A, NQ], f32, kind="ExternalInput").ap()
    out_d = nc.dram_tensor("out", [P, A, NQ], f32, kind="ExternalOutput").ap()

    with tile.TileContext(nc) as tc:
        with (
            tc.tile_pool(name="const", bufs=1) as const,
            tc.tile_pool(name="epool", bufs=8) as epool,
            tc.tile_pool(name="ipool", bufs=2) as ipool,
            tc.tile_pool(name="dpacc", bufs=2) as dpacc,
            tc.tile_pool(name="opool", bufs=4) as opool,
            tc.tile_pool(name="rpool", bufs=2) as rpool,
            tc.tile_pool(name="ps_st", bufs=2, space="PSUM") as ps_st,
            tc.tile_pool(name="ps_zq", bufs=1, space="PSUM") as ps_zq,
            tc.tile_pool(name="ps_small", bufs=1, space="PSUM") as ps_small,
            tc.tile_pool(name="dpool", bufs=2, space="DRAM") as dpool,
        ):
            # ---- persistent SBUF tensors ----
            mT_sb = const.tile([P, A, C], f8)
            x_sb = const.tile([P, A, NQ], f8)
            bw_sb = const.tile([P, A], f32)
            moTa_sb = const.tile([P, A, C], f8)
            y8_sb = const.tile([P, A, N], f8)
            bo2_sb = const.tile([P, A], f32)
            xres_sb = const.tile([P, A, NQ], f32)
            ones_row = const.tile([1, P], bf16)
            ones_col = const.tile([P, 1], bf16)
            ones_p2 = const.tile([P, 2, 16], f8)
            shift_sb = const.tile([P, 1], f32)
            qm_sb = const.tile([P, A, NQ], f8)
            vTo_sb = const.tile([P, JC, C], f8)

            # ---- loads, in order of first use (fat contiguous rows) ----
            nc.sync.dma_start(out=mT_sb, in_=mT_d)
            nc.sync.dma_start(out=x_sb, in_=x_d)
            nc.sync.dma_start(out=y8_sb[:, :, :N // 2], in_=y8_d[:, :, :N // 2])
            nc.sync.dma_start(out=moTa_sb, in_=moTa_d)
            nc.sync.dma_start(out=bw_sb, in_=bw_d)
            nc.sync.dma_start(out=y8_sb[:, :, N // 2:], in_=y8_d[:, :, N // 2:])
            nc.sync.dma_start(out=bo2_sb, in_=bo2_d)
            nc.sync.dma_start(out=xres_sb, in_=xres_d)
            nc.vector.memset(ones_row, 1.0)
            nc.vector.memset(ones_col, 1.0)
            nc.vector.memset(ones_p2, 1.0)
            nc.vector.memset(shift_sb, -2.5)
            # dummy exp so the ~1.3us ACT_TABLE_LOAD for the exp set runs
            # during the initial DMA wait instead of mid-projection
            warm_sb = const.tile([P, 1], f32)
            nc.scalar.activation(
                out=warm_sb, in_=shift_sb,
                func=mybir.ActivationFunctionType.Exp,
            )

            # ---- projection phase ----
            # PSUM->SBUF consumers alternate between ScalarE and VectorE so
            # neither engine serializes the phase (ScalarE is otherwise idle
            # until the exp stream starts), and the PSUM tiles cycle through
            # the stream's idle zq/rbc banks as well as the stp ring so ~5
            # units are in flight instead of 2.
            # 10:14 ACT:DVE — the stream start is gated by ScalarE draining
            # its projection share, while VectorE's two extra units spill
            # into the early stream's slack (consumers are needed only ~4
            # steps ahead there)
            eng_flip = [0]

            def consume(emit_act, emit_dve):
                i = eng_flip[0]
                (emit_act if (i % 2 == 0 and i not in (16, 20))
                 else emit_dve)()
                eng_flip[0] += 1

            proj_cycle = [(ps_st, "st"), (ps_zq, "zq0"), (ps_zq, "zq1"),
                          (ps_small, "rbcps"), (ps_small, "den")]
            proj_k = [0]

            def proj_tile(shape):
                pool, tag = proj_cycle[proj_k[0] % len(proj_cycle)]
                proj_k[0] += 1
                return pool.tile(shape, f32, name="proj_ps", tag=tag)

            # qm[c, l] = sum_c' M[c, c'] x[c', l] + bw
            def emit_qm(lt, och):
                lsl = slice(lt * LT, (lt + 1) * LT)
                ps = proj_tile([P, LT])
                nc.tensor.matmul(
                    ps,
                    mT_sb[:, :, och * P:(och + 1) * P],
                    x_sb[:, :, lsl],
                    start=True, stop=True, perf_mode=DR,
                )
                consume(
                    lambda: nc.scalar.activation(
                        out=qm_sb[:, och, lsl], in_=ps,
                        func=mybir.ActivationFunctionType.Identity,
                        bias=bw_sb[:, och:och + 1],
                    ),
                    lambda: nc.vector.tensor_scalar_add(
                        out=qm_sb[:, och, lsl], in0=ps,
                        scalar1=bw_sb[:, och:och + 1],
                    ),
                )

            # vTo[j, o] = sum_c y[c, j] MoT[c, o], two j-chunks per PSUM bank
            # so one cast moves both to SBUF
            def emit_vto_pair(k):
                ps = proj_tile([P, 2, C])
                for h in range(2):
                    nc.tensor.matmul(
                        ps[:, h, :],
                        y8_sb[:, :, (2 * k + h) * P:(2 * k + h + 1) * P],
                        moTa_sb[:, :, :],
                        start=True, stop=True, perf_mode=DR,
                    )
                src = ps.rearrange("p h c -> p (h c)")
                dst = vTo_sb[:, 2 * k:2 * k + 2, :].rearrange("p h c -> p (h c)")
                consume(
                    lambda: nc.scalar.activation(
                        out=dst, in_=src,
                        func=mybir.ActivationFunctionType.Copy,
                    ),
                    lambda: nc.vector.tensor_copy(out=dst, in_=src),
                )

            emit_qm(0, 0)
            emit_qm(0, 1)
            emit_vto_pair(0)
            emit_vto_pair(1)
            for lt in range(1, NLT):
                emit_qm(lt, 0)
                emit_qm(lt, 1)
                emit_vto_pair(2 * lt)
                emit_vto_pair(2 * lt + 1)
            for k in range(2 * NLT, JP):
                emit_vto_pair(k)

            # ---- attention stream, one-step software pipeline ----
            # prev = (lt, jp, e8) whose zq/den matmuls are not yet emitted;
            # pend = finished tile awaiting its deferred tail.
            out_r = out_d  # [P, A, NQ]

            def emit_zq_den(lt, jp, e8, zq, den, hsl=None):
                hs = hsl if hsl is not None else slice(0, LT)
                for m in range(A):
                    nc.tensor.matmul(
                        zq[m][:, hs],
                        vTo_sb[:, jp * 2:jp * 2 + 2, m * P:(m + 1) * P],
                        e8,
                        start=(jp == 0),
                        stop=(jp == JP - 1),
                        perf_mode=DR,
                    )
                if jp not in OFF_DEN:
                    # den's stop lands on the per-tile reduce matmul instead
                    nc.tensor.matmul(
                        den[:, hs],
                        ones_p2[:, :, 0:1],
                        e8,
                        start=(jp == 0),
                        stop=False,
                        perf_mode=DR,
                    )

            def emit_den_reduce(dacc, dacc_bf, den):
                # fold the VectorE-accumulated denominator partials into the
                # PSUM den row with a partition-reduce matmul pair (the bf16
                # cast of dacc was emitted early, at the end of its tile)
                for h in range(2):
                    nc.tensor.matmul(
                        den,
                        ones_col,
                        dacc_bf[:, h, :],
                        start=False,
                        stop=(h == 1),
                    )

            def emit_staging(lt, zq, den):
                # free the zq/den PSUM banks immediately: 1/den and raw zq
                # copies to SBUF; broadcast of 1/den via a DRAM round-trip
                # (fully off the PE/ACT critical path)
                r_sb = rpool.tile([1, LT], f32, tag="r")
                nc.vector.reciprocal_approx_fast(out=r_sb, in_=den)
                o_raw = []
                for och in range(A):
                    o = opool.tile([P, LT], f32, name="o_raw", tag=f"or{och}")
                    nc.vector.tensor_copy(out=o, in_=zq[och])
                    o_raw.append(o)
                r_dram = dpool.tile([1, LT], f32, name="r_dram", tag="rdram")
                nc.sync.dma_start(out=r_dram, in_=r_sb)
                rbc_sb = rpool.tile([P, LT], f32, tag="rbc")
                r_bcast_ap = bass.AP(
                    tensor=r_dram.tensor,
                    offset=r_dram.offset,
                    ap=[[0, P], list(r_dram.ap[-1])],
                )
                nc.sync.dma_start(out=rbc_sb, in_=r_bcast_ap)
                return (lt, o_raw, rbc_sb)

            def emit_tail(lt, o_raw, rbc_sb):
                tsl = slice(lt * LT, (lt + 1) * LT)
                for och in range(A):
                    o_sb = opool.tile([P, LT], f32, name="o_sb", tag="osb")
                    nc.vector.tensor_mul(
                        out=o_sb, in0=o_raw[och], in1=rbc_sb)
                    nc.vector.scalar_tensor_tensor(
                        out=o_sb,
                        in0=o_sb,
                        scalar=bo2_sb[:, och:och + 1],
                        in1=xres_sb[:, och, tsl],
                        op0=mybir.AluOpType.add,
                        op1=mybir.AluOpType.add,
                    )
                    nc.sync.dma_start(out=out_r[:, och, tsl], in_=o_sb)

            # three-deep deferral: the ST pair for step i+3 is emitted before
            # the zq/den consumers of step i, so the PE finishes the next
            # exp's inputs well before ScalarE needs them and the stream is
            # paced purely by the exp rate (the extra depth also covers the
            # longer latency of the VectorE-offloaded exps)
            prev = []
            pend = None
            for lt in range(NLT):
                lsl = slice(lt * LT, (lt + 1) * LT)
                zq0 = ps_zq.tile([P, LT], f32, tag="zq0")
                zq1 = ps_zq.tile([P, LT], f32, tag="zq1")
                zq_new = (zq0, zq1)
                den_new = ps_small.tile([1, LT], f32, tag="den")
                dacc = dpacc.tile([P, 2, LT], f32, name="dacc", tag="dacc")
                dacc_bf = dpacc.tile(
                    [P, 2, LT], bf16, name="dacc_bf", tag="daccbf")
                dacc_started = [False]

                def emit_dacc(src_ap, eng, dacc=dacc, started=dacc_started):
                    flat = dacc.rearrange("p h l -> p (h l)")
                    if not started[0]:
                        eng.tensor_copy(out=flat, in_=src_ap)
                        started[0] = True
                    else:
                        eng.tensor_add(out=flat, in0=flat, in1=src_ap)

                dacc_pend = None
                for jp in range(JP):
                    if dacc_pend is not None:
                        # den partial for an ACT-produced step, one full
                        # period later so the exp is surely done and the
                        # DVE FIFO never blocks on it
                        emit_dacc(dacc_pend, nc.vector)
                        dacc_pend = None
                    if jp == JP - 1:
                        # dacc is complete (last contributor flushed above);
                        # cast to bf16 now so the boundary's den-reduce
                        # matmuls never wait on VectorE
                        nc.vector.tensor_copy(out=dacc_bf, in_=dacc)
                    last_split = (lt == NLT - 1 and jp == JP - 1)
                    stp = ps_st.tile([P, 2, LT], f32, tag="st")
                    for h in range(2):
                        jc = jp * 2 + h
                        nc.tensor.matmul(
                            stp[:, h, :],
                            y8_sb[:, :, jc * P:(jc + 1) * P],
                            qm_sb[:, :, lsl],
                            start=True, stop=True, perf_mode=DR,
                        )
                    # -2.5 logit shift keeps exp within fp8e4 range (max
                    # +-240); it cancels exactly in the softmax
                    if not last_split:
                        if jp in OFF_EXP:
                            isb = ipool.tile([P, 2, LT], u8, name="isb")
                            nc.vector.tensor_scalar(
                                out=isb.rearrange("p h l -> p (h l)"),
                                in0=stp.rearrange("p h l -> p (h l)"),
                                scalar1=SCHRA_A,
                                scalar2=SCHRA_B,
                                op0=mybir.AluOpType.mult,
                                op1=mybir.AluOpType.add,
                            )
                            e8 = isb.bitcast(f8)
                            if jp in OFF_DEN:
                                dacc_pend = e8.rearrange("p h l -> p (h l)")
                        else:
                            e8 = epool.tile([P, 2, LT], f8)
                            nc.scalar.activation(
                                out=e8.rearrange("p h l -> p (h l)"),
                                in_=stp.rearrange("p h l -> p (h l)"),
                                func=mybir.ActivationFunctionType.Exp,
                                scale=float(SCALE),
                                bias=shift_sb,
                            )
                            if jp in OFF_DEN:
                                dacc_pend = e8.rearrange("p h l -> p (h l)")
                        if len(prev) == 4:
                            emit_zq_den(*prev.pop(0))
                        prev.append((lt, jp, e8, zq_new, den_new, None))
                    else:
                        # very last step: process in two l-halves so the
                        # serial softmax tail operates on half tiles and
                        # pipelines across engines
                        HL = LT // 2
                        for hh in range(2):
                            hsl = slice(hh * HL, (hh + 1) * HL)
                            e8h = epool.tile([P, 2, HL], f8)
                            nc.scalar.activation(
                                out=e8h,
                                in_=stp[:, :, hsl],
                                func=mybir.ActivationFunctionType.Exp,
                                scale=float(SCALE),
                                bias=shift_sb,
                            )
                            prev.append((lt, jp, e8h, zq_new, den_new, hsl))
                        while len(prev) > 2:
                            emit_zq_den(*prev.pop(0))
                    if lt > 0 and jp == 3:
                        # previous tile's accumulators just completed (its
                        # jp=15 consumers were emitted above): fold its
                        # VectorE den partials, then stage it out
                        emit_den_reduce(p_dacc, p_dacc_bf, last_den)
                        pend_new = emit_staging(lt - 1, last_zq, last_den)
                        if pend is not None:
                            emit_tail(*pend)
                        pend = pend_new
                last_zq, last_den = zq_new, den_new
                p_dacc, p_dacc_bf = dacc, dacc_bf

            # drain: the two final half-steps, the last deferred tail, and a
            # per-half low-latency tail (PE broadcast of 1/den)
            if pend is not None:
                emit_tail(*pend)
            rbc_ps = ps_small.tile([P, LT], f32, tag="rbcps")
            rbc_sb = rpool.tile([P, LT], f32, tag="rbc")
            HL = LT // 2
            for item in prev:
                emit_zq_den(*item)
            emit_den_reduce(p_dacc, p_dacc_bf, last_den)
            for item in prev:
                hsl = item[5]
                r_sb = rpool.tile([1, HL], f32, tag="r")
                nc.vector.reciprocal_approx_fast(
                    out=r_sb, in_=last_den[:, hsl])
                r_bf = rpool.tile([1, HL], bf16, tag="rbf")
                nc.vector.tensor_copy(out=r_bf, in_=r_sb)
                nc.tensor.matmul(
                    rbc_ps[:, hsl], ones_row, r_bf, start=True, stop=True)
                nc.scalar.activation(
                    out=rbc_sb[:, hsl], in_=rbc_ps[:, hsl],
                    func=mybir.ActivationFunctionType.Copy,
                )
                base = (NLT - 1) * LT
                tslh = slice(base + hsl.start, base + hsl.stop)
                for och in range(A):
                    o_sb = opool.tile([P, HL], f32, name="o_sb", tag="osb")
                    nc.vector.tensor_mul(
                        out=o_sb, in0=last_zq[och][:, hsl],
                        in1=rbc_sb[:, hsl])
                    nc.vector.scalar_tensor_tensor(
                        out=o_sb,
                        in0=o_sb,
                        scalar=bo2_sb[:, och:och + 1],
                        in1=xres_sb[:, och, tslh],
                        op0=mybir.AluOpType.add,
                        op1=mybir.AluOpType.add,
                    )
                    # final stores issue from otherwise-idle engine queues so
                    # the ~0.6us per-issue cost doesn't serialize on Sync
                    eng = [nc.gpsimd, nc.gpsimd, nc.sync, nc.scalar][
                        (0 if hsl.start == 0 else 2) + och]
                    eng.dma_start(out=out_r[:, och, tslh], in_=o_sb)

    nc.compile()
    return nc


_NC_CACHE = {}


def _use_v2(matmul_dt_name=MATMUL_DT, fp8_level=FP8_LEVEL):
    return matmul_dt_name == "bfloat16" and fp8_level >= 3 and not USE_V1


def _get_nc(key=None):
    if key is None:
        key = (MATMUL_DT, FP8_LEVEL)
    if key not in _NC_CACHE:
        if _use_v2(*key):
            _NC_CACHE[key] = build_fp8_v2()
        else:
            _NC_CACHE[key] = build_nc(*key)
    return _NC_CACHE[key]


def _shuf(arr, dt):
    """[C, w] -> partition-major [128, A, w] (c = a*128 + p)."""
    c, w = arr.shape
    return np.ascontiguousarray(
        arr.reshape(A, P, w).transpose(1, 0, 2)).astype(dt)


def make_in_maps_v2(x, y, Wq, bq, Wk, bk, Wv, bv, Wo, bo):
    import ml_dtypes

    f32, f64 = np.float32, np.float64
    f8np = ml_dtypes.float8_e4m3
    xf = np.asarray(x, f32).reshape(B, C, N)
    yf = np.asarray(y, f32).reshape(B, C, N)
    Wq64, Wk64, Wv64, Wo64 = (np.asarray(w, f64) for w in (Wq, Wk, Wv, Wo))
    bq64, bv64, bo64 = (np.asarray(b, f64) for b in (bq, bv, bo))
    mT = _shuf(np.ascontiguousarray((Wk64.T @ Wq64).T), f8np)
    moTa = _shuf(np.ascontiguousarray((Wo64 @ Wv64).T), f8np)
    bw = (Wk64.T @ bq64).astype(f32).reshape(A, P).T.copy()
    bo2 = (bo64 + Wo64 @ bv64).astype(f32).reshape(A, P).T.copy()
    y8c = np.clip(yf, -240, 240)
    in_maps = []
    for core in range(8):
        b, h = divmod(core, 2)
        xs = np.ascontiguousarray(xf[b][:, h * NQ:(h + 1) * NQ])
        in_maps.append({
            "x": _shuf(xs, f8np),
            "mT": mT, "moTa": moTa, "bw": bw, "bo2": bo2,
            "y8": _shuf(y8c[b], f8np),
            "xres": _shuf(xs, f32),
        })
    return in_maps


def make_in_maps(x, y, Wq, bq, Wk, bk, Wv, bv, Wo, bo,
                 matmul_dt_name: str = MATMUL_DT, fp8_level: int = FP8_LEVEL):
    if _use_v2(matmul_dt_name, fp8_level):
        return make_in_maps_v2(x, y, Wq, bq, Wk, bk, Wv, bv, Wo, bo)
    f32 = np.float32
    f64 = np.float64
    is_bf16 = matmul_dt_name == "bfloat16"
    st_fp8 = fp8_level >= 1 and is_bf16
    proj_fp8 = fp8_level >= 2 and is_bf16
    val_fp8 = fp8_level >= 3 and is_bf16
    if is_bf16:
        import ml_dtypes

        mnp = ml_dtypes.bfloat16
        f8np = ml_dtypes.float8_e4m3
    else:
        mnp = np.float32
        f8np = None
    xnp = f8np if proj_fp8 else mnp
    xf = np.asarray(x, f32).reshape(B, C, N)
    yf = np.asarray(y, f32).reshape(B, C, N)
    Wq64, Wk64, Wv64, Wo64 = (np.asarray(w, f64) for w in (Wq, Wk, Wv, Wo))
    bq64, bv64, bo64 = (np.asarray(b, f64) for b in (bq, bv, bo))
    mT = np.ascontiguousarray((Wk64.T @ Wq64).T).astype(xnp)
    moTa = np.ascontiguousarray((Wo64 @ Wv64).T).astype(xnp)
    bw = (Wk64.T @ bq64).astype(f32)
    bo2 = (bo64 + Wo64 @ bv64).astype(f32)
    if st_fp8:
        y8 = np.clip(yf, -240, 240).astype(f8np)
    in_maps = []
    for core in range(8):
        b, h = divmod(core, 2)
        xs = np.ascontiguousarray(xf[b][:, h * NQ:(h + 1) * NQ])
        m = {
            "x": xs.astype(xnp) if xnp is not np.float32 else xs,
            "mT": mT, "moTa": moTa,
            "bw": bw, "bo2": bo2,
        }
        if not proj_fp8:
            m["y"] = yf[b].astype(mnp) if mnp is not np.float32 else yf[b]
        if st_fp8:
            m["y8"] = y8[b]
        if is_bf16:
            m["xres"] = xs
        in_maps.append(m)
    return in_maps


def kernel(x, y, Wq, bq, Wk, bk, Wv, bv, Wo, bo):
    import contextlib

    import jax

    nc = _get_nc()
    in_maps = make_in_maps(x, y, Wq, bq, Wk, bk, Wv, bv, Wo, bo)
    # Pin the axon (NeuronCore) backend: run_bass_via_pjrt uses jax.devices(),
    # which follows the ambient default platform and silently miscomputes if a
    # caller set the default to CPU.
    try:
        axon_devs = jax.devices("axon")
    except RuntimeError:
        axon_devs = None
    ctx = jax.default_device(axon_devs[0]) if axon_devs else contextlib.nullcontext()
    with ctx:
        res = bass_utils.run_bass_kernel_spmd(nc, in_maps, core_ids=list(range(8)))
    v2 = _use_v2()
    out = np.empty((B, C, N), np.float32)
    for core in range(8):
        b, h = divmod(core, 2)
        o = np.asarray(res.results[core]["out"])
        if v2:  # [P, A, NQ] -> [C, NQ]
            o = o.transpose(1, 0, 2).reshape(C, NQ)
        out[b][:, h * NQ:(h + 1) * NQ] = o
    return out.reshape(B, C, H, W)



# revision 65
# speedup vs baseline: 1.0071x; 1.0071x over previous
"""Trainium2 Bass kernel for nn_Att_AdaIn (B=4, C=256, H=W=64 attention block).

Sharding: 8 cores = 4 batches x 2 query-halves. Each core holds the fused
weights, the full key/value source y[b] ([256, 4096]), and its own query
slice x[b][:, half] ([256, 2048]); it computes the full attention output for
its 2048 queries. Host gathers the 8 [256, 2048] results.

Weight fusion done on the host (in float64):
  logits: S = k^T q with q = Wq x + bq, k = Wk y + bk
        = y^T (Wk^T Wq) x + y^T (Wk^T bq) 1^T + [per-query-constant terms]
    The per-query-constant (l-only) terms are softmax-invariant and dropped.
    So with  M^T = (Wk^T Wq)^T  and  bw = Wk^T bq:   qm = M x + bw,
    ST[j,l] = sum_c y[c,j] qm[c,l].
  output: Wo (V E / den) + bo  with V = Wv y + bv 1^T
        = (Wo Wv) y E / den + Wo bv + bo
    So with MoT = (Wo Wv)^T and bo2 = bo + Wo bv, the value projection
    vTo = y^T MoT directly produces Wo-mixed values and the separate
    output projection disappears.

Per-core pipeline (layouts chosen so no on-chip transpose is needed):
  qm  = M x + bw               [c, l]      (c on partitions)
  vTo = y^T MoT                [j, 256]    (j on partitions)
  ST  = y^T qm                 [j, l]      (transposed attention scores)
  E   = exp(ST / sqrt(C))      (no max-subtraction: logits ~ N(0,1), fp32-safe)
  zq  = vTo^T E                [256, l]    unnormalized Wo-mixed output
  den = 1^T E                  [l]         softmax denominators (E summed on
                                           DVE, partition-reduced by one matmul)
  out = zq * (1/den) + bo2 + x

Dtype config via env:
  ATT_MATMUL_DT: float32 | float32r | bfloat16 (base matmul dtype)
  ATT_FP8: 0 = off (safest numerics, ~184 us, rel err ~3e-4),
           1 = fp8(e4m3) DoubleRow score matmuls (~129 us, rel err ~2e-3),
           2 = level 1 + fp8 DoubleRow qm/vTo projections,
           3 = level 2 + fp8 values/denominator with paired exps and a
               -2.5 logit shift (default; ~102.5 us, rel err ~4e-3).
  ATT_V1: 1 = use the older (pre-pipelined) emission for ATT_FP8=3.

The default (bf16 + ATT_FP8=3) path, measured at 99.8 us at full clock (vs
186.8 us for the session-start baseline; the chip sometimes sits in a ~1.2x
P0/thermal downclock — compare runs via the trace's exp duration, 1113 ns
at full clock), balances the 64-step stream across three engines:
 - per jc-pair step the PE issues fp8-DoubleRow matmuls (215 ns each at
   the N=512 hardware floor); ScalarE runs 14 of 16 exps per tile
   (1113 ns each); VectorE runs the other 2 via a log-domain Schraudolph
   exp — a single tensor_scalar producing fp8e4m3 BITS directly
   (uint8 = 8*(log2e*scaled_logit + 7 - 0.043), saturating at 0,
   bitcast to fp8; error ~ the fp8 quantization noise already on E) —
   and 7 of 16 denominator matmuls are replaced by VectorE f32
   accumulation folded back with one tiny partition-reduce matmul pair;
 - four-deep software pipeline: the score-matmul pair for step i+4 is
   emitted before the value/den consumers of step i, hiding both the exp
   latency and the longer VectorE-offload latency;
 - per-l-tile softmax tails (1/den, broadcast, final muls, store) are
   deferred into the next tile's stream via zq->SBUF staging copies +
   a DRAM-roundtrip broadcast of 1/den, keeping PSUM within 8 banks
   (stp ring 4 + zq 2 + den 1 + rbc 1);
 - the qm/vTo projection phase splits its PSUM->SBUF consumers
   alternately between ScalarE and VectorE and cycles PSUM tiles through
   the stream's idle zq/den/rbc banks (~6 units in flight);
 - the exp table load is prewarmed during the initial DMA wait; all DRAM
   tensors are pre-shuffled on the host into partition-major layout so
   every load is one fat contiguous-per-partition DMA; the final tile's
   tail is split into two l-halves whose output stores issue from three
   different engine queues.
"""

import os
import sys

for _p in ("/root/.axon_site", "/root/.axon_site/_ro/trn_rl_repo", "/opt/trn_rl_repo"):
    if os.path.isdir(_p) and _p not in sys.path:
        sys.path.append(_p)

import numpy as np

import concourse.bass as bass
from concourse import bacc, mybir, tile
from concourse import bass_utils

B, C, H, W = 4, 256, 64, 64
N = H * W          # 4096 pixels
NQ = N // 2        # 2048 queries per core
P = 128
A = C // P         # 2 channel chunks
LT = 512           # l-tile (query) width
NLT = NQ // LT     # 4 l-tiles
JC = N // P        # 32 key chunks
SCALE = 1.0 / np.sqrt(np.float32(C))  # 1/16

MATMUL_DT = os.environ.get("ATT_MATMUL_DT", "bfloat16")
FP8_LEVEL = int(os.environ.get("ATT_FP8", "3"))
USE_V1 = os.environ.get("ATT_V1", "0") == "1"


def build_nc(matmul_dt_name: str = MATMUL_DT, fp8_level: int = FP8_LEVEL):
    mdt = getattr(mybir.dt, matmul_dt_name)
    f32 = mybir.dt.float32
    f8 = mybir.dt.float8e4
    is_bf16 = mdt == mybir.dt.bfloat16
    st_fp8 = fp8_level >= 1 and is_bf16
    proj_fp8 = fp8_level >= 2 and is_bf16
    val_fp8 = fp8_level >= 3 and is_bf16
    DR = mybir.MatmulPerfMode.DoubleRow

    nc = bacc.Bacc("TRN2", target_bir_lowering=False, debug=False)

    # --- DRAM tensors ---
    xdt = f8 if proj_fp8 else mdt
    x_d = nc.dram_tensor("x", [C, NQ], xdt, kind="ExternalInput").ap()
    mT_d = nc.dram_tensor("mT", [C, C], xdt, kind="ExternalInput").ap()
    if st_fp8:
        y8_d = nc.dram_tensor("y8", [C, N], f8, kind="ExternalInput").ap()
    if not proj_fp8:
        y_d = nc.dram_tensor("y", [C, N], mdt, kind="ExternalInput").ap()
    moTa_d = nc.dram_tensor("moTa", [C, C], xdt, kind="ExternalInput").ap()
    bw_d = nc.dram_tensor("bw", [C], f32, kind="ExternalInput").ap()
    bo2_d = nc.dram_tensor("bo2", [C], f32, kind="ExternalInput").ap()
    if is_bf16:
        xres_d = nc.dram_tensor("xres", [C, NQ], f32, kind="ExternalInput").ap()
    out_d = nc.dram_tensor("out", [C, NQ], f32, kind="ExternalOutput").ap()

    qm_dt = f8 if st_fp8 else mdt

    with tile.TileContext(nc) as tc:
        with (
            tc.tile_pool(name="const", bufs=1) as const,
            tc.tile_pool(name="epool", bufs=8) as epool,
            tc.tile_pool(name="opool", bufs=3) as opool,
            tc.tile_pool(name="rpool", bufs=2) as rpool,
            tc.tile_pool(name="ps_st", bufs=2 if val_fp8 else 4, space="PSUM") as ps_st,
            tc.tile_pool(name="ps_zq", bufs=1 if val_fp8 else 2, space="PSUM") as ps_zq,
            tc.tile_pool(name="ps_small", bufs=1, space="PSUM") as ps_small,
            tc.tile_pool(name="dpool", bufs=2, space="DRAM") as dpool,
        ):
            # ---- persistent SBUF tensors ----
            x_sb = const.tile([P, A, NQ], xdt)
            mT_sb = const.tile([P, A, C], xdt)
            if st_fp8:
                y8_sb = const.tile([P, A, N], f8)
            if not proj_fp8:
                y_sb = const.tile([P, A, N], mdt)
            moTa_sb = const.tile([P, A, C], xdt)
            bw_sb = const.tile([P, A], f32)
            bo2_sb = const.tile([P, A], f32)
            ones_col = const.tile([P, 1], mdt)
            ones_row = const.tile([1, P], mdt)
            ones_p2 = const.tile([P, 2, 16], f8)
            shift_sb = const.tile([P, 1], f32)
            qm_sb = const.tile([P, A, NQ], qm_dt)
            vTo_sb = const.tile([P, JC, C], f8 if val_fp8 else mdt)
            if is_bf16:
                xres_sb = const.tile([P, A, NQ], f32)
            else:
                xres_sb = x_sb.bitcast(f32)

            # ---- loads (in order of first use; xres last, needed only at the end) ----
            xr_ = x_d.rearrange("(a p) n -> p a n", p=P)
            nc.sync.dma_start(out=x_sb[:, :, :NQ // 2], in_=xr_[:, :, :NQ // 2])
            nc.sync.dma_start(out=mT_sb, in_=mT_d.rearrange("(a p) o -> p a o", p=P))
            nc.sync.dma_start(out=bw_sb, in_=bw_d.rearrange("(a p) -> p a", p=P))
            nc.sync.dma_start(out=x_sb[:, :, NQ // 2:], in_=xr_[:, :, NQ // 2:])
            if not proj_fp8:
                yr_ = y_d.rearrange("(a p) n -> p a n", p=P)
                nc.sync.dma_start(out=y_sb[:, :, :N // 2], in_=yr_[:, :, :N // 2])
            nc.sync.dma_start(out=moTa_sb, in_=moTa_d.rearrange("(a p) o -> p a o", p=P))
            if not proj_fp8:
                nc.sync.dma_start(out=y_sb[:, :, N // 2:], in_=yr_[:, :, N // 2:])
            if st_fp8:
                y8r_ = y8_d.rearrange("(a p) n -> p a n", p=P)
                nc.sync.dma_start(out=y8_sb[:, :, :N // 2], in_=y8r_[:, :, :N // 2])
                nc.sync.dma_start(out=y8_sb[:, :, N // 2:], in_=y8r_[:, :, N // 2:])
            nc.sync.dma_start(out=bo2_sb, in_=bo2_d.rearrange("(a p) -> p a", p=P))
            nc.vector.memset(ones_col, 1.0)
            nc.vector.memset(ones_row, 1.0)
            nc.vector.memset(ones_p2, 1.0)
            nc.vector.memset(shift_sb, -2.5)
            if is_bf16:
                nc.sync.dma_start(
                    out=xres_sb, in_=xres_d.rearrange("(a p) n -> p a n", p=P)
                )

            # ---- projections ----
            # qm[c, l] = sum_c' M[c, c'] x[c', l] + bw[c]
            for lt in range(NLT):
                for och in range(A):
                    ps = ps_st.tile([P, LT], f32, tag="st")
                    if proj_fp8:
                        nc.tensor.matmul(
                            ps,
                            mT_sb[:, :, och * P:(och + 1) * P],
                            x_sb[:, :, lt * LT:(lt + 1) * LT],
                            start=True, stop=True, perf_mode=DR,
                        )
                    else:
                        for a in range(A):
                            nc.tensor.matmul(
                                ps,
                                mT_sb[:, a, och * P:(och + 1) * P],
                                x_sb[:, a, lt * LT:(lt + 1) * LT],
                                start=(a == 0),
                                stop=(a == A - 1),
                            )
                    nc.vector.tensor_scalar_add(
                        out=qm_sb[:, och, lt * LT:(lt + 1) * LT],
                        in0=ps,
                        scalar1=bw_sb[:, och:och + 1],
                    )
            # vTo[j, o] = sum_c y[c, j] MoT[c, o]
            # (for the generic path this is emitted inside the first attention
            # pass, one chunk ahead of its first use, so the PE stream never
            # stalls behind the y DMA; val_fp8 keeps the standalone loop)
            def emit_vto(jc):
                ps = ps_st.tile([P, C], f32, name="psv", tag="st")
                if proj_fp8:
                    nc.tensor.matmul(
                        ps,
                        y8_sb[:, :, jc * P:(jc + 1) * P],
                        moTa_sb[:, :, :],
                        start=True, stop=True, perf_mode=DR,
                    )
                else:
                    for a in range(A):
                        nc.tensor.matmul(
                            ps,
                            y_sb[:, a, jc * P:(jc + 1) * P],
                            moTa_sb[:, a, :],
                            start=(a == 0),
                            stop=(a == A - 1),
                        )
                nc.vector.tensor_copy(out=vTo_sb[:, jc, :], in_=ps)

            if st_fp8 or val_fp8:
                for jc in range(JC):
                    emit_vto(jc)

            # ---- attention, l-tile at a time ----
            def emit_tail(tl, zq, eacc):
                tsl = slice(tl * LT, (tl + 1) * LT)
                den_t = ps_st.tile([P, LT], f32, name="den_t", tag="st")
                den = den_t[0:1, :]
                nc.tensor.matmul(den, ones_col, eacc, start=True, stop=True)
                r_sb = rpool.tile([1, LT], f32, name="r_sb", tag="r")
                nc.vector.reciprocal_approx_fast(out=r_sb, in_=den)
                rbc_sb = rpool.tile([P, LT], f32, name="rbc_sb", tag="rbc")
                if tl == NLT - 1 and is_bf16:
                    # latency-critical final tile: broadcast r across
                    # partitions on the PE (bf16), skipping the DRAM trip
                    r_bf = rpool.tile([1, LT], mdt, name="r_bf", tag="rbf")
                    nc.vector.tensor_copy(out=r_bf, in_=r_sb)
                    rbc_ps = ps_st.tile([P, LT], f32, name="rbc_ps", tag="st")
                    nc.tensor.matmul(rbc_ps, ones_row, r_bf, start=True, stop=True)
                    nc.scalar.activation(
                        out=rbc_sb, in_=rbc_ps,
                        func=mybir.ActivationFunctionType.Copy,
                    )
                else:
                    # broadcast across partitions via a DRAM round-trip (off
                    # the PE/ACT critical path; overlapped by later matmuls)
                    r_dram = dpool.tile([1, LT], f32, name="r_dram", tag="rdram")
                    nc.sync.dma_start(out=r_dram, in_=r_sb)
                    r_bcast_ap = bass.AP(
                        tensor=r_dram.tensor,
                        offset=r_dram.offset,
                        ap=[[0, P], list(r_dram.ap[-1])],
                    )
                    nc.sync.dma_start(out=rbc_sb, in_=r_bcast_ap)
                for och in range(A):
                    o_sb = opool.tile([P, LT], f32, name="o_sb")
                    nc.vector.tensor_mul(out=o_sb, in0=zq[och], in1=rbc_sb)
                    nc.vector.scalar_tensor_tensor(
                        out=o_sb,
                        in0=o_sb,
                        scalar=bo2_sb[:, och:och + 1],
                        in1=xres_sb[:, och, tsl],
                        op0=mybir.AluOpType.add,
                        op1=mybir.AluOpType.add,
                    )
                    nc.sync.dma_start(
                        out=out_d.rearrange("(a p) n -> p a n", p=P)[:, och, tsl],
                        in_=o_sb,
                    )

            pending = None
            for lt in range(NLT):
                lsl = slice(lt * LT, (lt + 1) * LT)
                zq0 = ps_zq.tile([P, LT], f32, tag="zq0")
                zq1 = ps_zq.tile([P, LT], f32, tag="zq1")
                zq = (zq0, zq1)
                if val_fp8:
                    # fully fp8-DoubleRow attention: ST pairs -> one exp per
                    # pair -> DR value/denominator matmuls over jc-pairs
                    den = ps_small.tile([1, LT], f32, tag="den")
                    for jp in range(JC // 2):
                        stp = ps_st.tile([P, 2, LT], f32, tag="st")
                        for h in range(2):
                            jc = jp * 2 + h
                            nc.tensor.matmul(
                                stp[:, h, :],
                                y8_sb[:, :, jc * P:(jc + 1) * P],
                                qm_sb[:, :, lsl],
                                start=True, stop=True, perf_mode=DR,
                            )
                        e8 = epool.tile([P, 2, LT], f8)
                        # -2.5 logit shift keeps exp within fp8e4 range (max
                        # +-240); it scales numerator and denominator equally,
                        # so it cancels exactly in the softmax
                        nc.scalar.activation(
                            out=e8.rearrange("p h l -> p (h l)"),
                            in_=stp.rearrange("p h l -> p (h l)"),
                            func=mybir.ActivationFunctionType.Exp,
                            scale=float(SCALE),
                            bias=shift_sb,
                        )
                        for m in range(A):
                            nc.tensor.matmul(
                                zq[m],
                                vTo_sb[:, jp * 2:jp * 2 + 2, m * P:(m + 1) * P],
                                e8,
                                start=(jp == 0),
                                stop=(jp == JC // 2 - 1),
                                perf_mode=DR,
                            )
                        nc.tensor.matmul(
                            den,
                            ones_p2[:, :, 0:1],
                            e8,
                            start=(jp == 0),
                            stop=(jp == JC // 2 - 1),
                            perf_mode=DR,
                        )
                    r_sb = rpool.tile([1, LT], f32, tag="r")
                    nc.vector.reciprocal_approx_fast(out=r_sb, in_=den)
                    r_bf = rpool.tile([1, LT], mdt, tag="rbf")
                    nc.vector.tensor_copy(out=r_bf, in_=r_sb)
                    rbc_ps = ps_small.tile([P, LT], f32, tag="rbc")
                    nc.tensor.matmul(rbc_ps, ones_row, r_bf, start=True, stop=True)
                    rbc_sb = rpool.tile([P, LT], f32, tag="rbc")
                    nc.scalar.activation(
                        out=rbc_sb, in_=rbc_ps,
                        func=mybir.ActivationFunctionType.Copy,
                    )
                    for och in range(A):
                        o_sb = opool.tile([P, LT], f32)
                        nc.vector.tensor_mul(out=o_sb, in0=zq[och], in1=rbc_sb)
                        nc.vector.scalar_tensor_tensor(
                            out=o_sb,
                            in0=o_sb,
                            scalar=bo2_sb[:, och:och + 1],
                            in1=xres_sb[:, och, lsl],
                            op0=mybir.AluOpType.add,
                            op1=mybir.AluOpType.add,
                        )
                        nc.sync.dma_start(
                            out=out_d.rearrange("(a p) n -> p a n", p=P)[:, och, lsl],
                            in_=o_sb,
                        )
                    continue
                eacc = epool.tile([P, LT], mdt, tag="eacc")
                for jc in range(JC):
                    if lt == 0 and not st_fp8:
                        emit_vto(jc)
                    if jc == 3 and pending is not None:
                        emit_tail(*pending)
                        pending = None
                    st = ps_st.tile([P, LT], f32, tag="st")
                    if st_fp8:
                        nc.tensor.matmul(
                            st,
                            y8_sb[:, :, jc * P:(jc + 1) * P],
                            qm_sb[:, :, lsl],
                            start=True, stop=True, perf_mode=DR,
                        )
                    else:
                        for a in range(A):
                            nc.tensor.matmul(
                                st,
                                y_sb[:, a, jc * P:(jc + 1) * P],
                                qm_sb[:, a, lsl],
                                start=(a == 0),
                                stop=(a == A - 1),
                            )
                    e_sb = epool.tile([P, LT], mdt)
                    nc.scalar.activation(
                        out=e_sb,
                        in_=st,
                        func=mybir.ActivationFunctionType.Exp,
                        scale=float(SCALE),
                    )
                    for m in range(A):
                        nc.tensor.matmul(
                            zq[m],
                            vTo_sb[:, jc, m * P:(m + 1) * P],
                            e_sb,
                            start=(jc == 0),
                            stop=(jc == JC - 1),
                        )
                    if jc == 0:
                        nc.vector.tensor_copy(out=eacc, in_=e_sb)
                    else:
                        nc.vector.tensor_add(out=eacc, in0=eacc, in1=e_sb)

                # tail (den reduce -> 1/den -> broadcast -> finals) is
                # deferred into the next tile pass so the den matmul's wait on
                # the DVE accumulation chain can't stall the PE stream at the
                # tile boundary
                pending = (lt, zq, eacc)
            if pending is not None:
                emit_tail(*pending)

    nc.compile()
    return nc


def build_fp8_v2():
    """Optimized bf16+ATT_FP8=3 kernel: software-pipelined PE emission,
    deferred softmax tails, partition-major single-DMA loads."""
    f32 = mybir.dt.float32
    bf16 = mybir.dt.bfloat16
    u8 = mybir.dt.uint8
    f8 = mybir.dt.float8e4
    DR = mybir.MatmulPerfMode.DoubleRow
    JP = JC // 2  # 16 jc-pairs per l-tile

    # three-engine rebalance: ScalarE exp (1113ns) and the PE's 5 matmuls
    # (1075ns) are otherwise both ~99% busy per step. Two exps per tile run
    # on VectorE instead via a Schraudolph bit-trick (i32 = A*x + B
    # reinterpreted as f32; ~2% elementwise error, below the fp8-e4m3
    # quantization noise already on E), and five denominator matmuls per
    # tile are replaced by VectorE f32 accumulation folded back with one
    # tiny partition-reduce matmul pair.
    OFF_EXP = (5, 11)
    # NOTE: this exact set is load-bearing for schedule quality — both
    # clustering the flushes after the offload slots (5,6,7,11,12,13,14)
    # and swapping 3->7 measured slower; the residual ~1us/tile of ScalarE
    # stalls after the offload slots did not respond to flush placement
    OFF_DEN = (2, 3, 5, 6, 8, 11, 13)
    # log-domain Schraudolph straight to fp8e4m3 bits: uint8 out =
    # 8*(log2(e)*x + 7 - 0.043), saturating at 0 for underflow; bitcast to
    # fp8. One VectorE op replaces exp+cast.
    LOG2E = 1.4426950408889634
    SCHRA_A = float(8 * LOG2E * SCALE)
    SCHRA_B = float(8 * 7 - 8 * 0.0430 - 8 * LOG2E * 2.5)

    nc = bacc.Bacc("TRN2", target_bir_lowering=False, debug=False)

    # --- DRAM tensors, pre-shuffled on host to partition-major layout ---
    mT_d = nc.dram_tensor("mT", [P, A, C], f8, kind="ExternalInput").ap()
    x_d = nc.dram_tensor("x", [P, A, NQ], f8, kind="ExternalInput").ap()
    bw_d = nc.dram_tensor("bw", [P, A], f32, kind="ExternalInput").ap()
    moTa_d = nc.dram_tensor("moTa", [P, A, C], f8, kind="ExternalInput").ap()
    y8_d = nc.dram_tensor("y8", [P, A, N], f8, kind="ExternalInput").ap()
    bo2_d = nc.dram_tensor("bo2", [P, A], f32, kind="ExternalInput").ap()
    xres_d = nc.dram_tensor("xres", [P, A, NQ], f32, kind="ExternalInput").ap()
    out_d = nc.dram_tensor("out", [P, A, NQ], f32, kind="ExternalOutput").ap()

    with tile.TileContext(nc) as tc:
        with (
            tc.tile_pool(name="const", bufs=1) as const,
            # 16-deep: the dacc flush adds read e8 tiles on VectorE well
            # after the zq consumers; an 8-ring made new exps WAR-wait on
            # flushes from 8 steps earlier (~0.8-2us ScalarE stalls)
            tc.tile_pool(name="epool", bufs=16) as epool,
            tc.tile_pool(name="ipool", bufs=2) as ipool,
            tc.tile_pool(name="dpacc", bufs=2) as dpacc,
            tc.tile_pool(name="opool", bufs=4) as opool,
            tc.tile_pool(name="rpool", bufs=2) as rpool,
            tc.tile_pool(name="ps_st", bufs=2, space="PSUM") as ps_st,
            tc.tile_pool(name="ps_zq", bufs=1, space="PSUM") as ps_zq,
            tc.tile_pool(name="ps_small", bufs=1, space="PSUM") as ps_small,
            tc.tile_pool(name="dpool", bufs=2, space="DRAM") as dpool,
        ):
            # ---- persistent SBUF tensors ----
            mT_sb = const.tile([P, A, C], f8)
            x_sb = const.tile([P, A, NQ], f8)
            bw_sb = const.tile([P, A], f32)
            moTa_sb = const.tile([P, A, C], f8)
            y8_sb = const.tile([P, A, N], f8)
            bo2_sb = const.tile([P, A], f32)
            xres_sb = const.tile([P, A, NQ], f32)
            ones_row = const.tile([1, P], bf16)
            ones_col = const.tile([P, 1], bf16)
            ones_p2 = const.tile([P, 2, 16], f8)
            shift_sb = const.tile([P, 1], f32)
            qm_sb = const.tile([P, A, NQ], f8)
            vTo_sb = const.tile([P, JC, C], f8)

            # ---- loads, in order of first use (fat contiguous rows) ----
            nc.sync.dma_start(out=mT_sb, in_=mT_d)
            nc.sync.dma_start(out=x_sb, in_=x_d)
            nc.sync.dma_start(out=y8_sb[:, :, :N // 2], in_=y8_d[:, :, :N // 2])
            nc.sync.dma_start(out=moTa_sb, in_=moTa_d)
            nc.sync.dma_start(out=bw_sb, in_=bw_d)
            nc.sync.dma_start(out=y8_sb[:, :, N // 2:], in_=y8_d[:, :, N // 2:])
            nc.sync.dma_start(out=bo2_sb, in_=bo2_d)
            nc.sync.dma_start(out=xres_sb, in_=xres_d)
            nc.vector.memset(ones_row, 1.0)
            nc.vector.memset(ones_col, 1.0)
            nc.vector.memset(ones_p2, 1.0)
            nc.vector.memset(shift_sb, -2.5)
            # dummy exp so the ~1.3us ACT_TABLE_LOAD for the exp set runs
            # during the initial DMA wait instead of mid-projection
            warm_sb = const.tile([P, 1], f32)
            nc.scalar.activation(
                out=warm_sb, in_=shift_sb,
                func=mybir.ActivationFunctionType.Exp,
            )

            # ---- projection phase ----
            # PSUM->SBUF consumers alternate between ScalarE and VectorE so
            # neither engine serializes the phase (ScalarE is otherwise idle
            # until the exp stream starts), and the PSUM tiles cycle through
            # the stream's idle zq/rbc banks as well as the stp ring so ~5
            # units are in flight instead of 2.
            # 10:14 ACT:DVE — the stream start is gated by ScalarE draining
            # its projection share, while VectorE's two extra units spill
            # into the early stream's slack (consumers are needed only ~4
            # steps ahead there)
            eng_flip = [0]

            def consume(emit_act, emit_dve):
                i = eng_flip[0]
                (emit_act if (i % 2 == 0 and i not in (16, 20))
                 else emit_dve)()
                eng_flip[0] += 1

            proj_cycle = [(ps_st, "st"), (ps_zq, "zq0"), (ps_zq, "zq1"),
                          (ps_small, "rbcps"), (ps_small, "den")]
            proj_k = [0]

            def proj_tile(shape):
                pool, tag = proj_cycle[proj_k[0] % len(proj_cycle)]
                proj_k[0] += 1
                return pool.tile(shape, f32, name="proj_ps", tag=tag)

            # qm[c, l] = sum_c' M[c, c'] x[c', l] + bw
            def emit_qm(lt, och):
                lsl = slice(lt * LT, (lt + 1) * LT)
                ps = proj_tile([P, LT])
                nc.tensor.matmul(
                    ps,
                    mT_sb[:, :, och * P:(och + 1) * P],
                    x_sb[:, :, lsl],
                    start=True, stop=True, perf_mode=DR,
                )
                consume(
                    lambda: nc.scalar.activation(
                        out=qm_sb[:, och, lsl], in_=ps,
                        func=mybir.ActivationFunctionType.Identity,
                        bias=bw_sb[:, och:och + 1],
                    ),
                    lambda: nc.vector.tensor_scalar_add(
                        out=qm_sb[:, och, lsl], in0=ps,
                        scalar1=bw_sb[:, och:och + 1],
                    ),
                )

            # vTo[j, o] = sum_c y[c, j] MoT[c, o], two j-chunks per PSUM bank
            # so one cast moves both to SBUF
            def emit_vto_pair(k):
                ps = proj_tile([P, 2, C])
                for h in range(2):
                    nc.tensor.matmul(
                        ps[:, h, :],
                        y8_sb[:, :, (2 * k + h) * P:(2 * k + h + 1) * P],
                        moTa_sb[:, :, :],
                        start=True, stop=True, perf_mode=DR,
                    )
                src = ps.rearrange("p h c -> p (h c)")
                dst = vTo_sb[:, 2 * k:2 * k + 2, :].rearrange("p h c -> p (h c)")
                consume(
                    lambda: nc.scalar.activation(
                        out=dst, in_=src,
                        func=mybir.ActivationFunctionType.Copy,
                    ),
                    lambda: nc.vector.tensor_copy(out=dst, in_=src),
                )

            emit_qm(0, 0)
            emit_qm(0, 1)
            emit_vto_pair(0)
            emit_vto_pair(1)
            for lt in range(1, NLT):
                emit_qm(lt, 0)
                emit_qm(lt, 1)
                emit_vto_pair(2 * lt)
                emit_vto_pair(2 * lt + 1)
            for k in range(2 * NLT, JP):
                emit_vto_pair(k)

            # ---- attention stream, one-step software pipeline ----
            # prev = (lt, jp, e8) whose zq/den matmuls are not yet emitted;
            # pend = finished tile awaiting its deferred tail.
            out_r = out_d  # [P, A, NQ]

            def emit_zq_den(lt, jp, e8, zq, den, hsl=None):
                hs = hsl if hsl is not None else slice(0, LT)
                for m in range(A):
                    nc.tensor.matmul(
                        zq[m][:, hs],
                        vTo_sb[:, jp * 2:jp * 2 + 2, m * P:(m + 1) * P],
                        e8,
                        start=(jp == 0),
                        stop=(jp == JP - 1),
                        perf_mode=DR,
                    )
                if jp not in OFF_DEN:
                    # den's stop lands on the per-tile reduce matmul instead
                    nc.tensor.matmul(
                        den[:, hs],
                        ones_p2[:, :, 0:1],
                        e8,
                        start=(jp == 0),
                        stop=False,
                        perf_mode=DR,
                    )

            def emit_den_reduce(dacc, dacc_bf, den):
                # fold the VectorE-accumulated denominator partials into the
                # PSUM den row with a partition-reduce matmul pair (the bf16
                # cast of dacc was emitted early, at the end of its tile)
                for h in range(2):
                    nc.tensor.matmul(
                        den,
                        ones_col,
                        dacc_bf[:, h, :],
                        start=False,
                        stop=(h == 1),
                    )

            def emit_staging(lt, zq, den):
                # free the zq/den PSUM banks immediately: 1/den and raw zq
                # copies to SBUF; broadcast of 1/den via a DRAM round-trip
                # (fully off the PE/ACT critical path)
                r_sb = rpool.tile([1, LT], f32, tag="r")
                nc.vector.reciprocal_approx_fast(out=r_sb, in_=den)
                o_raw = []
                for och in range(A):
                    o = opool.tile([P, LT], f32, name="o_raw", tag=f"or{och}")
                    nc.vector.tensor_copy(out=o, in_=zq[och])
                    o_raw.append(o)
                r_dram = dpool.tile([1, LT], f32, name="r_dram", tag="rdram")
                nc.sync.dma_start(out=r_dram, in_=r_sb)
                rbc_sb = rpool.tile([P, LT], f32, tag="rbc")
                r_bcast_ap = bass.AP(
                    tensor=r_dram.tensor,
                    offset=r_dram.offset,
                    ap=[[0, P], list(r_dram.ap[-1])],
                )
                nc.sync.dma_start(out=rbc_sb, in_=r_bcast_ap)
                return (lt, o_raw, rbc_sb)

            def emit_tail(lt, o_raw, rbc_sb):
                tsl = slice(lt * LT, (lt + 1) * LT)
                for och in range(A):
                    o_sb = opool.tile([P, LT], f32, name="o_sb", tag="osb")
                    nc.vector.tensor_mul(
                        out=o_sb, in0=o_raw[och], in1=rbc_sb)
                    nc.vector.scalar_tensor_tensor(
                        out=o_sb,
                        in0=o_sb,
                        scalar=bo2_sb[:, och:och + 1],
                        in1=xres_sb[:, och, tsl],
                        op0=mybir.AluOpType.add,
                        op1=mybir.AluOpType.add,
                    )
                    nc.sync.dma_start(out=out_r[:, och, tsl], in_=o_sb)

            # three-deep deferral: the ST pair for step i+3 is emitted before
            # the zq/den consumers of step i, so the PE finishes the next
            # exp's inputs well before ScalarE needs them and the stream is
            # paced purely by the exp rate (the extra depth also covers the
            # longer latency of the VectorE-offloaded exps)
            prev = []
            pend = None
            for lt in range(NLT):
                lsl = slice(lt * LT, (lt + 1) * LT)
                zq0 = ps_zq.tile([P, LT], f32, tag="zq0")
                zq1 = ps_zq.tile([P, LT], f32, tag="zq1")
                zq_new = (zq0, zq1)
                den_new = ps_small.tile([1, LT], f32, tag="den")
                dacc = dpacc.tile([P, 2, LT], f32, name="dacc", tag="dacc")
                dacc_bf = dpacc.tile(
                    [P, 2, LT], bf16, name="dacc_bf", tag="daccbf")
                dacc_started = [False]

                def emit_dacc(src_ap, eng, dacc=dacc, started=dacc_started):
                    flat = dacc.rearrange("p h l -> p (h l)")
                    if not started[0]:
                        eng.tensor_copy(out=flat, in_=src_ap)
                        started[0] = True
                    else:
                        eng.tensor_add(out=flat, in0=flat, in1=src_ap)

                dacc_pend = None
                for jp in range(JP):
                    if dacc_pend is not None:
                        # den partial for an ACT-produced step, one full
                        # period later so the exp is surely done and the
                        # DVE FIFO never blocks on it
                        emit_dacc(dacc_pend, nc.vector)
                        dacc_pend = None
                    if jp == JP - 1:
                        # dacc is complete (last contributor flushed above);
                        # cast to bf16 now so the boundary's den-reduce
                        # matmuls never wait on VectorE
                        nc.vector.tensor_copy(out=dacc_bf, in_=dacc)
                    last_split = (lt == NLT - 1 and jp == JP - 1)
                    stp = ps_st.tile([P, 2, LT], f32, tag="st")
                    for h in range(2):
                        jc = jp * 2 + h
                        nc.tensor.matmul(
                            stp[:, h, :],
                            y8_sb[:, :, jc * P:(jc + 1) * P],
                            qm_sb[:, :, lsl],
                            start=True, stop=True, perf_mode=DR,
                        )
                    # -2.5 logit shift keeps exp within fp8e4 range (max
                    # +-240); it cancels exactly in the softmax
                    if not last_split:
                        if jp in OFF_EXP:
                            isb = ipool.tile([P, 2, LT], u8, name="isb")
                            nc.vector.tensor_scalar(
                                out=isb.rearrange("p h l -> p (h l)"),
                                in0=stp.rearrange("p h l -> p (h l)"),
                                scalar1=SCHRA_A,
                                scalar2=SCHRA_B,
                                op0=mybir.AluOpType.mult,
                                op1=mybir.AluOpType.add,
                            )
                            e8 = isb.bitcast(f8)
                            if jp in OFF_DEN:
                                dacc_pend = e8.rearrange("p h l -> p (h l)")
                        else:
                            e8 = epool.tile([P, 2, LT], f8)
                            nc.scalar.activation(
                                out=e8.rearrange("p h l -> p (h l)"),
                                in_=stp.rearrange("p h l -> p (h l)"),
                                func=mybir.ActivationFunctionType.Exp,
                                scale=float(SCALE),
                                bias=shift_sb,
                            )
                            if jp in OFF_DEN:
                                dacc_pend = e8.rearrange("p h l -> p (h l)")
                        if len(prev) == 4:
                            emit_zq_den(*prev.pop(0))
                        prev.append((lt, jp, e8, zq_new, den_new, None))
                    else:
                        # very last step: process in two l-halves so the
                        # serial softmax tail operates on half tiles and
                        # pipelines across engines
                        HL = LT // 2
                        for hh in range(2):
                            hsl = slice(hh * HL, (hh + 1) * HL)
                            e8h = epool.tile([P, 2, HL], f8)
                            nc.scalar.activation(
                                out=e8h,
                                in_=stp[:, :, hsl],
                                func=mybir.ActivationFunctionType.Exp,
                                scale=float(SCALE),
                                bias=shift_sb,
                            )
                            prev.append((lt, jp, e8h, zq_new, den_new, hsl))
                        while len(prev) > 2:
                            emit_zq_den(*prev.pop(0))
                    if lt > 0 and jp == 3:
                        # previous tile's accumulators just completed (its
                        # jp=15 consumers were emitted above): fold its
                        # VectorE den partials, then stage it out
                        emit_den_reduce(p_dacc, p_dacc_bf, last_den)
                        pend_new = emit_staging(lt - 1, last_zq, last_den)
                        if pend is not None:
                            emit_tail(*pend)
                        pend = pend_new
                last_zq, last_den = zq_new, den_new
                p_dacc, p_dacc_bf = dacc, dacc_bf

            # drain: the two final half-steps, the last deferred tail, and a
            # per-half low-latency tail (PE broadcast of 1/den)
            if pend is not None:
                emit_tail(*pend)
            rbc_ps = ps_small.tile([P, LT], f32, tag="rbcps")
            rbc_sb = rpool.tile([P, LT], f32, tag="rbc")
            HL = LT // 2
            for item in prev:
                emit_zq_den(*item)
            emit_den_reduce(p_dacc, p_dacc_bf, last_den)
            for item in prev:
                hsl = item[5]
                r_sb = rpool.tile([1, HL], f32, tag="r")
                nc.vector.reciprocal_approx_fast(
                    out=r_sb, in_=last_den[:, hsl])
                r_bf = rpool.tile([1, HL], bf16, tag="rbf")
                nc.vector.tensor_copy(out=r_bf, in_=r_sb)
                nc.tensor.matmul(
                    rbc_ps[:, hsl], ones_row, r_bf, start=True, stop=True)
                nc.scalar.activation(
                    out=rbc_sb[:, hsl], in_=rbc_ps[:, hsl],
                    func=mybir.ActivationFunctionType.Copy,
                )
                base = (NLT - 1) * LT
                tslh = slice(base + hsl.start, base + hsl.stop)
                for och in range(A):
                    o_sb = opool.tile([P, HL], f32, name="o_sb", tag="osb")
                    nc.vector.tensor_mul(
                        out=o_sb, in0=last_zq[och][:, hsl],
                        in1=rbc_sb[:, hsl])
                    nc.vector.scalar_tensor_tensor(
                        out=o_sb,
                        in0=o_sb,
                        scalar=bo2_sb[:, och:och + 1],
                        in1=xres_sb[:, och, tslh],
                        op0=mybir.AluOpType.add,
                        op1=mybir.AluOpType.add,
                    )
                    # final stores issue from otherwise-idle engine queues so
                    # the ~0.6us per-issue cost doesn't serialize on Sync
                    eng = [nc.gpsimd, nc.gpsimd, nc.sync, nc.scalar][
                        (0 if hsl.start == 0 else 2) + och]
                    eng.dma_start(out=out_r[:, och, tslh], in_=o_sb)

    nc.compile()
    return nc


_NC_CACHE = {}


def _use_v2(matmul_dt_name=MATMUL_DT, fp8_level=FP8_LEVEL):
    return matmul_dt_name == "bfloat16" and fp8_level >= 3 and not USE_V1


def _get_nc(key=None):
    if key is None:
        key = (MATMUL_DT, FP8_LEVEL)
    if key not in _NC_CACHE:
        if _use_v2(*key):
            _NC_CACHE[key] = build_fp8_v2()
        else:
            _NC_CACHE[key] = build_nc(*key)
    return _NC_CACHE[key]


def _shuf(arr, dt):
    """[C, w] -> partition-major [128, A, w] (c = a*128 + p)."""
    c, w = arr.shape
    return np.ascontiguousarray(
        arr.reshape(A, P, w).transpose(1, 0, 2)).astype(dt)


def make_in_maps_v2(x, y, Wq, bq, Wk, bk, Wv, bv, Wo, bo):
    import ml_dtypes

    f32, f64 = np.float32, np.float64
    f8np = ml_dtypes.float8_e4m3
    xf = np.asarray(x, f32).reshape(B, C, N)
    yf = np.asarray(y, f32).reshape(B, C, N)
    Wq64, Wk64, Wv64, Wo64 = (np.asarray(w, f64) for w in (Wq, Wk, Wv, Wo))
    bq64, bv64, bo64 = (np.asarray(b, f64) for b in (bq, bv, bo))
    mT = _shuf(np.ascontiguousarray((Wk64.T @ Wq64).T), f8np)
    moTa = _shuf(np.ascontiguousarray((Wo64 @ Wv64).T), f8np)
    bw = (Wk64.T @ bq64).astype(f32).reshape(A, P).T.copy()
    bo2 = (bo64 + Wo64 @ bv64).astype(f32).reshape(A, P).T.copy()
    y8c = np.clip(yf, -240, 240)
    in_maps = []
    for core in range(8):
        b, h = divmod(core, 2)
        xs = np.ascontiguousarray(xf[b][:, h * NQ:(h + 1) * NQ])
        in_maps.append({
            "x": _shuf(xs, f8np),
            "mT": mT, "moTa": moTa, "bw": bw, "bo2": bo2,
            "y8": _shuf(y8c[b], f8np),
            "xres": _shuf(xs, f32),
        })
    return in_maps


def make_in_maps(x, y, Wq, bq, Wk, bk, Wv, bv, Wo, bo,
                 matmul_dt_name: str = MATMUL_DT, fp8_level: int = FP8_LEVEL):
    if _use_v2(matmul_dt_name, fp8_level):
        return make_in_maps_v2(x, y, Wq, bq, Wk, bk, Wv, bv, Wo, bo)
    f32 = np.float32
    f64 = np.float64
    is_bf16 = matmul_dt_name == "bfloat16"
    st_fp8 = fp8_level >= 1 and is_bf16
    proj_fp8 = fp8_level >= 2 and is_bf16
    val_fp8 = fp8_level >= 3 and is_bf16
    if is_bf16:
        import ml_dtypes

        mnp = ml_dtypes.bfloat16
        f8np = ml_dtypes.float8_e4m3
    else:
        mnp = np.float32
        f8np = None
    xnp = f8np if proj_fp8 else mnp
    xf = np.asarray(x, f32).reshape(B, C, N)
    yf = np.asarray(y, f32).reshape(B, C, N)
    Wq64, Wk64, Wv64, Wo64 = (np.asarray(w, f64) for w in (Wq, Wk, Wv, Wo))
    bq64, bv64, bo64 = (np.asarray(b, f64) for b in (bq, bv, bo))
    mT = np.ascontiguousarray((Wk64.T @ Wq64).T).astype(xnp)
    moTa = np.ascontiguousarray((Wo64 @ Wv64).T).astype(xnp)
    bw = (Wk64.T @ bq64).astype(f32)
    bo2 = (bo64 + Wo64 @ bv64).astype(f32)
    if st_fp8:
        y8 = np.clip(yf, -240, 240).astype(f8np)
    in_maps = []
    for core in range(8):
        b, h = divmod(core, 2)
        xs = np.ascontiguousarray(xf[b][:, h * NQ:(h + 1) * NQ])
        m = {
            "x": xs.astype(xnp) if xnp is not np.float32 else xs,
            "mT": mT, "moTa": moTa,
            "bw": bw, "bo2": bo2,
        }
        if not proj_fp8:
            m["y"] = yf[b].astype(mnp) if mnp is not np.float32 else yf[b]
        if st_fp8:
            m["y8"] = y8[b]
        if is_bf16:
            m["xres"] = xs
        in_maps.append(m)
    return in_maps


def kernel(x, y, Wq, bq, Wk, bk, Wv, bv, Wo, bo):
    import contextlib

    import jax

    nc = _get_nc()
    in_maps = make_in_maps(x, y, Wq, bq, Wk, bk, Wv, bv, Wo, bo)
    # Pin the axon (NeuronCore) backend: run_bass_via_pjrt uses jax.devices(),
    # which follows the ambient default platform and silently miscomputes if a
    # caller set the default to CPU.
    try:
        axon_devs = jax.devices("axon")
    except RuntimeError:
        axon_devs = None
    ctx = jax.default_device(axon_devs[0]) if axon_devs else contextlib.nullcontext()
    with ctx:
        res = bass_utils.run_bass_kernel_spmd(nc, in_maps, core_ids=list(range(8)))
    v2 = _use_v2()
    out = np.empty((B, C, N), np.float32)
    for core in range(8):
        b, h = divmod(core, 2)
        o = np.asarray(res.results[core]["out"])
        if v2:  # [P, A, NQ] -> [C, NQ]
            o = o.transpose(1, 0, 2).reshape(C, NQ)
        out[b][:, h * NQ:(h + 1) * NQ] = o
    return out.reshape(B, C, H, W)



# revision 68
# speedup vs baseline: 1.0107x; 1.0035x over previous
"""Trainium2 Bass kernel for nn_Att_AdaIn (B=4, C=256, H=W=64 attention block).

Sharding: 8 cores = 4 batches x 2 query-halves. Each core holds the fused
weights, the full key/value source y[b] ([256, 4096]), and its own query
slice x[b][:, half] ([256, 2048]); it computes the full attention output for
its 2048 queries. Host gathers the 8 [256, 2048] results.

Weight fusion done on the host (in float64):
  logits: S = k^T q with q = Wq x + bq, k = Wk y + bk
        = y^T (Wk^T Wq) x + y^T (Wk^T bq) 1^T + [per-query-constant terms]
    The per-query-constant (l-only) terms are softmax-invariant and dropped.
    So with  M^T = (Wk^T Wq)^T  and  bw = Wk^T bq:   qm = M x + bw,
    ST[j,l] = sum_c y[c,j] qm[c,l].
  output: Wo (V E / den) + bo  with V = Wv y + bv 1^T
        = (Wo Wv) y E / den + Wo bv + bo
    So with MoT = (Wo Wv)^T and bo2 = bo + Wo bv, the value projection
    vTo = y^T MoT directly produces Wo-mixed values and the separate
    output projection disappears.

Per-core pipeline (layouts chosen so no on-chip transpose is needed):
  qm  = M x + bw               [c, l]      (c on partitions)
  vTo = y^T MoT                [j, 256]    (j on partitions)
  ST  = y^T qm                 [j, l]      (transposed attention scores)
  E   = exp(ST / sqrt(C))      (no max-subtraction: logits ~ N(0,1), fp32-safe)
  zq  = vTo^T E                [256, l]    unnormalized Wo-mixed output
  den = 1^T E                  [l]         softmax denominators (E summed on
                                           DVE, partition-reduced by one matmul)
  out = zq * (1/den) + bo2 + x

Dtype config via env:
  ATT_MATMUL_DT: float32 | float32r | bfloat16 (base matmul dtype)
  ATT_FP8: 0 = off (safest numerics, ~184 us, rel err ~3e-4),
           1 = fp8(e4m3) DoubleRow score matmuls (~129 us, rel err ~2e-3),
           2 = level 1 + fp8 DoubleRow qm/vTo projections,
           3 = level 2 + fp8 values/denominator with paired exps and a
               -2.5 logit shift (default; ~102.5 us, rel err ~4e-3).
  ATT_V1: 1 = use the older (pre-pipelined) emission for ATT_FP8=3.

The default (bf16 + ATT_FP8=3) path, measured at 99.8 us at full clock (vs
186.8 us for the session-start baseline; the chip sometimes sits in a ~1.2x
P0/thermal downclock — compare runs via the trace's exp duration, 1113 ns
at full clock), balances the 64-step stream across three engines:
 - per jc-pair step the PE issues fp8-DoubleRow matmuls (215 ns each at
   the N=512 hardware floor); ScalarE runs 14 of 16 exps per tile
   (1113 ns each); VectorE runs the other 2 via a log-domain Schraudolph
   exp — a single tensor_scalar producing fp8e4m3 BITS directly
   (uint8 = 8*(log2e*scaled_logit + 7 - 0.043), saturating at 0,
   bitcast to fp8; error ~ the fp8 quantization noise already on E) —
   and 7 of 16 denominator matmuls are replaced by VectorE f32
   accumulation folded back with one tiny partition-reduce matmul pair;
 - four-deep software pipeline: the score-matmul pair for step i+4 is
   emitted before the value/den consumers of step i, hiding both the exp
   latency and the longer VectorE-offload latency;
 - per-l-tile softmax tails (1/den, broadcast, final muls, store) are
   deferred into the next tile's stream via zq->SBUF staging copies +
   a DRAM-roundtrip broadcast of 1/den, keeping PSUM within 8 banks
   (stp ring 4 + zq 2 + den 1 + rbc 1);
 - the qm/vTo projection phase splits its PSUM->SBUF consumers
   alternately between ScalarE and VectorE and cycles PSUM tiles through
   the stream's idle zq/den/rbc banks (~6 units in flight);
 - the exp table load is prewarmed during the initial DMA wait; all DRAM
   tensors are pre-shuffled on the host into partition-major layout so
   every load is one fat contiguous-per-partition DMA; the final tile's
   tail is split into two l-halves whose output stores issue from three
   different engine queues.
"""

import os
import sys

for _p in ("/root/.axon_site", "/root/.axon_site/_ro/trn_rl_repo", "/opt/trn_rl_repo"):
    if os.path.isdir(_p) and _p not in sys.path:
        sys.path.append(_p)

import numpy as np

import concourse.bass as bass
from concourse import bacc, mybir, tile
from concourse import bass_utils

B, C, H, W = 4, 256, 64, 64
N = H * W          # 4096 pixels
NQ = N // 2        # 2048 queries per core
P = 128
A = C // P         # 2 channel chunks
LT = 512           # l-tile (query) width
NLT = NQ // LT     # 4 l-tiles
JC = N // P        # 32 key chunks
SCALE = 1.0 / np.sqrt(np.float32(C))  # 1/16

MATMUL_DT = os.environ.get("ATT_MATMUL_DT", "bfloat16")
FP8_LEVEL = int(os.environ.get("ATT_FP8", "3"))
USE_V1 = os.environ.get("ATT_V1", "0") == "1"


def build_nc(matmul_dt_name: str = MATMUL_DT, fp8_level: int = FP8_LEVEL):
    mdt = getattr(mybir.dt, matmul_dt_name)
    f32 = mybir.dt.float32
    f8 = mybir.dt.float8e4
    is_bf16 = mdt == mybir.dt.bfloat16
    st_fp8 = fp8_level >= 1 and is_bf16
    proj_fp8 = fp8_level >= 2 and is_bf16
    val_fp8 = fp8_level >= 3 and is_bf16
    DR = mybir.MatmulPerfMode.DoubleRow

    nc = bacc.Bacc("TRN2", target_bir_lowering=False, debug=False)

    # --- DRAM tensors ---
    xdt = f8 if proj_fp8 else mdt
    x_d = nc.dram_tensor("x", [C, NQ], xdt, kind="ExternalInput").ap()
    mT_d = nc.dram_tensor("mT", [C, C], xdt, kind="ExternalInput").ap()
    if st_fp8:
        y8_d = nc.dram_tensor("y8", [C, N], f8, kind="ExternalInput").ap()
    if not proj_fp8:
        y_d = nc.dram_tensor("y", [C, N], mdt, kind="ExternalInput").ap()
    moTa_d = nc.dram_tensor("moTa", [C, C], xdt, kind="ExternalInput").ap()
    bw_d = nc.dram_tensor("bw", [C], f32, kind="ExternalInput").ap()
    bo2_d = nc.dram_tensor("bo2", [C], f32, kind="ExternalInput").ap()
    if is_bf16:
        xres_d = nc.dram_tensor("xres", [C, NQ], f32, kind="ExternalInput").ap()
    out_d = nc.dram_tensor("out", [C, NQ], f32, kind="ExternalOutput").ap()

    qm_dt = f8 if st_fp8 else mdt

    with tile.TileContext(nc) as tc:
        with (
            tc.tile_pool(name="const", bufs=1) as const,
            tc.tile_pool(name="epool", bufs=8) as epool,
            tc.tile_pool(name="opool", bufs=3) as opool,
            tc.tile_pool(name="rpool", bufs=2) as rpool,
            tc.tile_pool(name="ps_st", bufs=2 if val_fp8 else 4, space="PSUM") as ps_st,
            tc.tile_pool(name="ps_zq", bufs=1 if val_fp8 else 2, space="PSUM") as ps_zq,
            tc.tile_pool(name="ps_small", bufs=1, space="PSUM") as ps_small,
            tc.tile_pool(name="dpool", bufs=2, space="DRAM") as dpool,
        ):
            # ---- persistent SBUF tensors ----
            x_sb = const.tile([P, A, NQ], xdt)
            mT_sb = const.tile([P, A, C], xdt)
            if st_fp8:
                y8_sb = const.tile([P, A, N], f8)
            if not proj_fp8:
                y_sb = const.tile([P, A, N], mdt)
            moTa_sb = const.tile([P, A, C], xdt)
            bw_sb = const.tile([P, A], f32)
            bo2_sb = const.tile([P, A], f32)
            ones_col = const.tile([P, 1], mdt)
            ones_row = const.tile([1, P], mdt)
            ones_p2 = const.tile([P, 2, 16], f8)
            shift_sb = const.tile([P, 1], f32)
            qm_sb = const.tile([P, A, NQ], qm_dt)
            vTo_sb = const.tile([P, JC, C], f8 if val_fp8 else mdt)
            if is_bf16:
                xres_sb = const.tile([P, A, NQ], f32)
            else:
                xres_sb = x_sb.bitcast(f32)

            # ---- loads (in order of first use; xres last, needed only at the end) ----
            xr_ = x_d.rearrange("(a p) n -> p a n", p=P)
            nc.sync.dma_start(out=x_sb[:, :, :NQ // 2], in_=xr_[:, :, :NQ // 2])
            nc.sync.dma_start(out=mT_sb, in_=mT_d.rearrange("(a p) o -> p a o", p=P))
            nc.sync.dma_start(out=bw_sb, in_=bw_d.rearrange("(a p) -> p a", p=P))
            nc.sync.dma_start(out=x_sb[:, :, NQ // 2:], in_=xr_[:, :, NQ // 2:])
            if not proj_fp8:
                yr_ = y_d.rearrange("(a p) n -> p a n", p=P)
                nc.sync.dma_start(out=y_sb[:, :, :N // 2], in_=yr_[:, :, :N // 2])
            nc.sync.dma_start(out=moTa_sb, in_=moTa_d.rearrange("(a p) o -> p a o", p=P))
            if not proj_fp8:
                nc.sync.dma_start(out=y_sb[:, :, N // 2:], in_=yr_[:, :, N // 2:])
            if st_fp8:
                y8r_ = y8_d.rearrange("(a p) n -> p a n", p=P)
                nc.sync.dma_start(out=y8_sb[:, :, :N // 2], in_=y8r_[:, :, :N // 2])
                nc.sync.dma_start(out=y8_sb[:, :, N // 2:], in_=y8r_[:, :, N // 2:])
            nc.sync.dma_start(out=bo2_sb, in_=bo2_d.rearrange("(a p) -> p a", p=P))
            nc.vector.memset(ones_col, 1.0)
            nc.vector.memset(ones_row, 1.0)
            nc.vector.memset(ones_p2, 1.0)
            nc.vector.memset(shift_sb, -2.5)
            if is_bf16:
                nc.sync.dma_start(
                    out=xres_sb, in_=xres_d.rearrange("(a p) n -> p a n", p=P)
                )

            # ---- projections ----
            # qm[c, l] = sum_c' M[c, c'] x[c', l] + bw[c]
            for lt in range(NLT):
                for och in range(A):
                    ps = ps_st.tile([P, LT], f32, tag="st")
                    if proj_fp8:
                        nc.tensor.matmul(
                            ps,
                            mT_sb[:, :, och * P:(och + 1) * P],
                            x_sb[:, :, lt * LT:(lt + 1) * LT],
                            start=True, stop=True, perf_mode=DR,
                        )
                    else:
                        for a in range(A):
                            nc.tensor.matmul(
                                ps,
                                mT_sb[:, a, och * P:(och + 1) * P],
                                x_sb[:, a, lt * LT:(lt + 1) * LT],
                                start=(a == 0),
                                stop=(a == A - 1),
                            )
                    nc.vector.tensor_scalar_add(
                        out=qm_sb[:, och, lt * LT:(lt + 1) * LT],
                        in0=ps,
                        scalar1=bw_sb[:, och:och + 1],
                    )
            # vTo[j, o] = sum_c y[c, j] MoT[c, o]
            # (for the generic path this is emitted inside the first attention
            # pass, one chunk ahead of its first use, so the PE stream never
            # stalls behind the y DMA; val_fp8 keeps the standalone loop)
            def emit_vto(jc):
                ps = ps_st.tile([P, C], f32, name="psv", tag="st")
                if proj_fp8:
                    nc.tensor.matmul(
                        ps,
                        y8_sb[:, :, jc * P:(jc + 1) * P],
                        moTa_sb[:, :, :],
                        start=True, stop=True, perf_mode=DR,
                    )
                else:
                    for a in range(A):
                        nc.tensor.matmul(
                            ps,
                            y_sb[:, a, jc * P:(jc + 1) * P],
                            moTa_sb[:, a, :],
                            start=(a == 0),
                            stop=(a == A - 1),
                        )
                nc.vector.tensor_copy(out=vTo_sb[:, jc, :], in_=ps)

            if st_fp8 or val_fp8:
                for jc in range(JC):
                    emit_vto(jc)

            # ---- attention, l-tile at a time ----
            def emit_tail(tl, zq, eacc):
                tsl = slice(tl * LT, (tl + 1) * LT)
                den_t = ps_st.tile([P, LT], f32, name="den_t", tag="st")
                den = den_t[0:1, :]
                nc.tensor.matmul(den, ones_col, eacc, start=True, stop=True)
                r_sb = rpool.tile([1, LT], f32, name="r_sb", tag="r")
                nc.vector.reciprocal_approx_fast(out=r_sb, in_=den)
                rbc_sb = rpool.tile([P, LT], f32, name="rbc_sb", tag="rbc")
                if tl == NLT - 1 and is_bf16:
                    # latency-critical final tile: broadcast r across
                    # partitions on the PE (bf16), skipping the DRAM trip
                    r_bf = rpool.tile([1, LT], mdt, name="r_bf", tag="rbf")
                    nc.vector.tensor_copy(out=r_bf, in_=r_sb)
                    rbc_ps = ps_st.tile([P, LT], f32, name="rbc_ps", tag="st")
                    nc.tensor.matmul(rbc_ps, ones_row, r_bf, start=True, stop=True)
                    nc.scalar.activation(
                        out=rbc_sb, in_=rbc_ps,
                        func=mybir.ActivationFunctionType.Copy,
                    )
                else:
                    # broadcast across partitions via a DRAM round-trip (off
                    # the PE/ACT critical path; overlapped by later matmuls)
                    r_dram = dpool.tile([1, LT], f32, name="r_dram", tag="rdram")
                    nc.sync.dma_start(out=r_dram, in_=r_sb)
                    r_bcast_ap = bass.AP(
                        tensor=r_dram.tensor,
                        offset=r_dram.offset,
                        ap=[[0, P], list(r_dram.ap[-1])],
                    )
                    nc.sync.dma_start(out=rbc_sb, in_=r_bcast_ap)
                for och in range(A):
                    o_sb = opool.tile([P, LT], f32, name="o_sb")
                    nc.vector.tensor_mul(out=o_sb, in0=zq[och], in1=rbc_sb)
                    nc.vector.scalar_tensor_tensor(
                        out=o_sb,
                        in0=o_sb,
                        scalar=bo2_sb[:, och:och + 1],
                        in1=xres_sb[:, och, tsl],
                        op0=mybir.AluOpType.add,
                        op1=mybir.AluOpType.add,
                    )
                    nc.sync.dma_start(
                        out=out_d.rearrange("(a p) n -> p a n", p=P)[:, och, tsl],
                        in_=o_sb,
                    )

            pending = None
            for lt in range(NLT):
                lsl = slice(lt * LT, (lt + 1) * LT)
                zq0 = ps_zq.tile([P, LT], f32, tag="zq0")
                zq1 = ps_zq.tile([P, LT], f32, tag="zq1")
                zq = (zq0, zq1)
                if val_fp8:
                    # fully fp8-DoubleRow attention: ST pairs -> one exp per
                    # pair -> DR value/denominator matmuls over jc-pairs
                    den = ps_small.tile([1, LT], f32, tag="den")
                    for jp in range(JC // 2):
                        stp = ps_st.tile([P, 2, LT], f32, tag="st")
                        for h in range(2):
                            jc = jp * 2 + h
                            nc.tensor.matmul(
                                stp[:, h, :],
                                y8_sb[:, :, jc * P:(jc + 1) * P],
                                qm_sb[:, :, lsl],
                                start=True, stop=True, perf_mode=DR,
                            )
                        e8 = epool.tile([P, 2, LT], f8)
                        # -2.5 logit shift keeps exp within fp8e4 range (max
                        # +-240); it scales numerator and denominator equally,
                        # so it cancels exactly in the softmax
                        nc.scalar.activation(
                            out=e8.rearrange("p h l -> p (h l)"),
                            in_=stp.rearrange("p h l -> p (h l)"),
                            func=mybir.ActivationFunctionType.Exp,
                            scale=float(SCALE),
                            bias=shift_sb,
                        )
                        for m in range(A):
                            nc.tensor.matmul(
                                zq[m],
                                vTo_sb[:, jp * 2:jp * 2 + 2, m * P:(m + 1) * P],
                                e8,
                                start=(jp == 0),
                                stop=(jp == JC // 2 - 1),
                                perf_mode=DR,
                            )
                        nc.tensor.matmul(
                            den,
                            ones_p2[:, :, 0:1],
                            e8,
                            start=(jp == 0),
                            stop=(jp == JC // 2 - 1),
                            perf_mode=DR,
                        )
                    r_sb = rpool.tile([1, LT], f32, tag="r")
                    nc.vector.reciprocal_approx_fast(out=r_sb, in_=den)
                    r_bf = rpool.tile([1, LT], mdt, tag="rbf")
                    nc.vector.tensor_copy(out=r_bf, in_=r_sb)
                    rbc_ps = ps_small.tile([P, LT], f32, tag="rbc")
                    nc.tensor.matmul(rbc_ps, ones_row, r_bf, start=True, stop=True)
                    rbc_sb = rpool.tile([P, LT], f32, tag="rbc")
                    nc.scalar.activation(
                        out=rbc_sb, in_=rbc_ps,
                        func=mybir.ActivationFunctionType.Copy,
                    )
                    for och in range(A):
                        o_sb = opool.tile([P, LT], f32)
                        nc.vector.tensor_mul(out=o_sb, in0=zq[och], in1=rbc_sb)
                        nc.vector.scalar_tensor_tensor(
                            out=o_sb,
                            in0=o_sb,
                            scalar=bo2_sb[:, och:och + 1],
                            in1=xres_sb[:, och, lsl],
                            op0=mybir.AluOpType.add,
                            op1=mybir.AluOpType.add,
                        )
                        nc.sync.dma_start(
                            out=out_d.rearrange("(a p) n -> p a n", p=P)[:, och, lsl],
                            in_=o_sb,
                        )
                    continue
                eacc = epool.tile([P, LT], mdt, tag="eacc")
                for jc in range(JC):
                    if lt == 0 and not st_fp8:
                        emit_vto(jc)
                    if jc == 3 and pending is not None:
                        emit_tail(*pending)
                        pending = None
                    st = ps_st.tile([P, LT], f32, tag="st")
                    if st_fp8:
                        nc.tensor.matmul(
                            st,
                            y8_sb[:, :, jc * P:(jc + 1) * P],
                            qm_sb[:, :, lsl],
                            start=True, stop=True, perf_mode=DR,
                        )
                    else:
                        for a in range(A):
                            nc.tensor.matmul(
                                st,
                                y_sb[:, a, jc * P:(jc + 1) * P],
                                qm_sb[:, a, lsl],
                                start=(a == 0),
                                stop=(a == A - 1),
                            )
                    e_sb = epool.tile([P, LT], mdt)
                    nc.scalar.activation(
                        out=e_sb,
                        in_=st,
                        func=mybir.ActivationFunctionType.Exp,
                        scale=float(SCALE),
                    )
                    for m in range(A):
                        nc.tensor.matmul(
                            zq[m],
                            vTo_sb[:, jc, m * P:(m + 1) * P],
                            e_sb,
                            start=(jc == 0),
                            stop=(jc == JC - 1),
                        )
                    if jc == 0:
                        nc.vector.tensor_copy(out=eacc, in_=e_sb)
                    else:
                        nc.vector.tensor_add(out=eacc, in0=eacc, in1=e_sb)

                # tail (den reduce -> 1/den -> broadcast -> finals) is
                # deferred into the next tile pass so the den matmul's wait on
                # the DVE accumulation chain can't stall the PE stream at the
                # tile boundary
                pending = (lt, zq, eacc)
            if pending is not None:
                emit_tail(*pending)

    nc.compile()
    return nc


def build_fp8_v2():
    """Optimized bf16+ATT_FP8=3 kernel: software-pipelined PE emission,
    deferred softmax tails, partition-major single-DMA loads."""
    f32 = mybir.dt.float32
    bf16 = mybir.dt.bfloat16
    u8 = mybir.dt.uint8
    f8 = mybir.dt.float8e4
    DR = mybir.MatmulPerfMode.DoubleRow
    JP = JC // 2  # 16 jc-pairs per l-tile

    # three-engine rebalance: ScalarE exp (1113ns) and the PE's 5 matmuls
    # (1075ns) are otherwise both ~99% busy per step. Two exps per tile run
    # on VectorE instead via a Schraudolph bit-trick (i32 = A*x + B
    # reinterpreted as f32; ~2% elementwise error, below the fp8-e4m3
    # quantization noise already on E), and five denominator matmuls per
    # tile are replaced by VectorE f32 accumulation folded back with one
    # tiny partition-reduce matmul pair.
    OFF_EXP = (5, 11)
    # NOTE: this exact set is load-bearing for schedule quality — both
    # clustering the flushes after the offload slots (5,6,7,11,12,13,14)
    # and swapping 3->7 measured slower; the residual ~1us/tile of ScalarE
    # stalls after the offload slots did not respond to flush placement
    OFF_DEN = (2, 3, 5, 6, 8, 11, 13)
    # log-domain Schraudolph straight to fp8e4m3 bits: uint8 out =
    # 8*(log2(e)*x + 7 - 0.043), saturating at 0 for underflow; bitcast to
    # fp8. One VectorE op replaces exp+cast.
    LOG2E = 1.4426950408889634
    SCHRA_A = float(8 * LOG2E * SCALE)
    SCHRA_B = float(8 * 7 - 8 * 0.0430 - 8 * LOG2E * 2.5)

    nc = bacc.Bacc("TRN2", target_bir_lowering=False, debug=False)

    # --- DRAM tensors, pre-shuffled on host to partition-major layout ---
    # mT and x are host-concatenated into one tensor: one DMA issue
    # (~620ns) less ahead of the critical path, earlier qm inputs
    mx_d = nc.dram_tensor("mx", [P, A, C + NQ], f8, kind="ExternalInput").ap()
    bw_d = nc.dram_tensor("bw", [P, A], f32, kind="ExternalInput").ap()
    moTa_d = nc.dram_tensor("moTa", [P, A, C], f8, kind="ExternalInput").ap()
    y8_d = nc.dram_tensor("y8", [P, A, N], f8, kind="ExternalInput").ap()
    bo2_d = nc.dram_tensor("bo2", [P, A], f32, kind="ExternalInput").ap()
    xres_d = nc.dram_tensor("xres", [P, A, NQ], f32, kind="ExternalInput").ap()
    out_d = nc.dram_tensor("out", [P, A, NQ], f32, kind="ExternalOutput").ap()

    with tile.TileContext(nc) as tc:
        with (
            tc.tile_pool(name="const", bufs=1) as const,
            # 16-deep: the dacc flush adds read e8 tiles on VectorE well
            # after the zq consumers; an 8-ring made new exps WAR-wait on
            # flushes from 8 steps earlier (~0.8-2us ScalarE stalls)
            tc.tile_pool(name="epool", bufs=16) as epool,
            tc.tile_pool(name="ipool", bufs=2) as ipool,
            tc.tile_pool(name="dpacc", bufs=2) as dpacc,
            tc.tile_pool(name="opool", bufs=4) as opool,
            tc.tile_pool(name="rpool", bufs=2) as rpool,
            tc.tile_pool(name="ps_st", bufs=2, space="PSUM") as ps_st,
            tc.tile_pool(name="ps_zq", bufs=1, space="PSUM") as ps_zq,
            tc.tile_pool(name="ps_small", bufs=1, space="PSUM") as ps_small,
            tc.tile_pool(name="dpool", bufs=2, space="DRAM") as dpool,
        ):
            # ---- persistent SBUF tensors ----
            mx_sb = const.tile([P, A, C + NQ], f8)
            mT_sb = mx_sb[:, :, :C]
            x_sb = mx_sb[:, :, C:]
            bw_sb = const.tile([P, A], f32)
            moTa_sb = const.tile([P, A, C], f8)
            y8_sb = const.tile([P, A, N], f8)
            bo2_sb = const.tile([P, A], f32)
            xres_sb = const.tile([P, A, NQ], f32)
            ones_row = const.tile([1, P], bf16)
            ones_col = const.tile([P, 1], bf16)
            ones_p2 = const.tile([P, 2, 16], f8)
            shift_sb = const.tile([P, 1], f32)
            qm_sb = const.tile([P, A, NQ], f8)
            vTo_sb = const.tile([P, JC, C], f8)

            # ---- loads, in order of first use (fat contiguous rows) ----
            nc.sync.dma_start(out=mx_sb, in_=mx_d)
            nc.sync.dma_start(out=y8_sb[:, :, :N // 2], in_=y8_d[:, :, :N // 2])
            nc.sync.dma_start(out=moTa_sb, in_=moTa_d)
            nc.sync.dma_start(out=bw_sb, in_=bw_d)
            nc.sync.dma_start(out=y8_sb[:, :, N // 2:], in_=y8_d[:, :, N // 2:])
            nc.sync.dma_start(out=bo2_sb, in_=bo2_d)
            nc.sync.dma_start(out=xres_sb, in_=xres_d)
            nc.vector.memset(ones_row, 1.0)
            nc.vector.memset(ones_col, 1.0)
            nc.vector.memset(ones_p2, 1.0)
            nc.vector.memset(shift_sb, -2.5)
            # dummy exp so the ~1.3us ACT_TABLE_LOAD for the exp set runs
            # during the initial DMA wait instead of mid-projection
            warm_sb = const.tile([P, 1], f32)
            nc.scalar.activation(
                out=warm_sb, in_=shift_sb,
                func=mybir.ActivationFunctionType.Exp,
            )

            # ---- projection phase ----
            # PSUM->SBUF consumers alternate between ScalarE and VectorE so
            # neither engine serializes the phase (ScalarE is otherwise idle
            # until the exp stream starts), and the PSUM tiles cycle through
            # the stream's idle zq/rbc banks as well as the stp ring so ~5
            # units are in flight instead of 2.
            # 10:14 ACT:DVE — the stream start is gated by ScalarE draining
            # its projection share, while VectorE's two extra units spill
            # into the early stream's slack (consumers are needed only ~4
            # steps ahead there)
            eng_flip = [0]

            def consume(emit_act, emit_dve):
                i = eng_flip[0]
                (emit_act if (i % 2 == 0 and i not in (16, 20))
                 else emit_dve)()
                eng_flip[0] += 1

            proj_cycle = [(ps_st, "st"), (ps_zq, "zq0"), (ps_zq, "zq1"),
                          (ps_small, "rbcps"), (ps_small, "den")]
            proj_k = [0]

            def proj_tile(shape):
                pool, tag = proj_cycle[proj_k[0] % len(proj_cycle)]
                proj_k[0] += 1
                return pool.tile(shape, f32, name="proj_ps", tag=tag)

            # qm[c, l] = sum_c' M[c, c'] x[c', l] + bw
            def emit_qm(lt, och):
                lsl = slice(lt * LT, (lt + 1) * LT)
                ps = proj_tile([P, LT])
                nc.tensor.matmul(
                    ps,
                    mT_sb[:, :, och * P:(och + 1) * P],
                    x_sb[:, :, lsl],
                    start=True, stop=True, perf_mode=DR,
                )
                consume(
                    lambda: nc.scalar.activation(
                        out=qm_sb[:, och, lsl], in_=ps,
                        func=mybir.ActivationFunctionType.Identity,
                        bias=bw_sb[:, och:och + 1],
                    ),
                    lambda: nc.vector.tensor_scalar_add(
                        out=qm_sb[:, och, lsl], in0=ps,
                        scalar1=bw_sb[:, och:och + 1],
                    ),
                )

            # vTo[j, o] = sum_c y[c, j] MoT[c, o], two j-chunks per PSUM bank
            # so one cast moves both to SBUF
            def emit_vto_pair(k):
                ps = proj_tile([P, 2, C])
                for h in range(2):
                    nc.tensor.matmul(
                        ps[:, h, :],
                        y8_sb[:, :, (2 * k + h) * P:(2 * k + h + 1) * P],
                        moTa_sb[:, :, :],
                        start=True, stop=True, perf_mode=DR,
                    )
                src = ps.rearrange("p h c -> p (h c)")
                dst = vTo_sb[:, 2 * k:2 * k + 2, :].rearrange("p h c -> p (h c)")
                consume(
                    lambda: nc.scalar.activation(
                        out=dst, in_=src,
                        func=mybir.ActivationFunctionType.Copy,
                    ),
                    lambda: nc.vector.tensor_copy(out=dst, in_=src),
                )

            emit_qm(0, 0)
            emit_qm(0, 1)
            emit_vto_pair(0)
            emit_vto_pair(1)
            for lt in range(1, NLT):
                emit_qm(lt, 0)
                emit_qm(lt, 1)
                emit_vto_pair(2 * lt)
                emit_vto_pair(2 * lt + 1)
            for k in range(2 * NLT, JP):
                emit_vto_pair(k)

            # ---- attention stream, one-step software pipeline ----
            # prev = (lt, jp, e8) whose zq/den matmuls are not yet emitted;
            # pend = finished tile awaiting its deferred tail.
            out_r = out_d  # [P, A, NQ]

            def emit_zq_den(lt, jp, e8, zq, den, hsl=None):
                hs = hsl if hsl is not None else slice(0, LT)
                for m in range(A):
                    nc.tensor.matmul(
                        zq[m][:, hs],
                        vTo_sb[:, jp * 2:jp * 2 + 2, m * P:(m + 1) * P],
                        e8,
                        start=(jp == 0),
                        stop=(jp == JP - 1),
                        perf_mode=DR,
                    )
                if jp not in OFF_DEN:
                    # den's stop lands on the per-tile reduce matmul instead
                    nc.tensor.matmul(
                        den[:, hs],
                        ones_p2[:, :, 0:1],
                        e8,
                        start=(jp == 0),
                        stop=False,
                        perf_mode=DR,
                    )

            def emit_den_reduce(dacc, dacc_bf, den):
                # fold the VectorE-accumulated denominator partials into the
                # PSUM den row with a partition-reduce matmul pair (the bf16
                # cast of dacc was emitted early, at the end of its tile)
                for h in range(2):
                    nc.tensor.matmul(
                        den,
                        ones_col,
                        dacc_bf[:, h, :],
                        start=False,
                        stop=(h == 1),
                    )

            def emit_staging(lt, zq, den):
                # free the zq/den PSUM banks immediately: 1/den and raw zq
                # copies to SBUF; broadcast of 1/den via a DRAM round-trip
                # (fully off the PE/ACT critical path)
                r_sb = rpool.tile([1, LT], f32, tag="r")
                nc.vector.reciprocal_approx_fast(out=r_sb, in_=den)
                o_raw = []
                for och in range(A):
                    o = opool.tile([P, LT], f32, name="o_raw", tag=f"or{och}")
                    nc.vector.tensor_copy(out=o, in_=zq[och])
                    o_raw.append(o)
                r_dram = dpool.tile([1, LT], f32, name="r_dram", tag="rdram")
                nc.sync.dma_start(out=r_dram, in_=r_sb)
                rbc_sb = rpool.tile([P, LT], f32, tag="rbc")
                r_bcast_ap = bass.AP(
                    tensor=r_dram.tensor,
                    offset=r_dram.offset,
                    ap=[[0, P], list(r_dram.ap[-1])],
                )
                nc.sync.dma_start(out=rbc_sb, in_=r_bcast_ap)
                return (lt, o_raw, rbc_sb)

            def emit_tail(lt, o_raw, rbc_sb):
                tsl = slice(lt * LT, (lt + 1) * LT)
                for och in range(A):
                    o_sb = opool.tile([P, LT], f32, name="o_sb", tag="osb")
                    nc.vector.tensor_mul(
                        out=o_sb, in0=o_raw[och], in1=rbc_sb)
                    nc.vector.scalar_tensor_tensor(
                        out=o_sb,
                        in0=o_sb,
                        scalar=bo2_sb[:, och:och + 1],
                        in1=xres_sb[:, och, tsl],
                        op0=mybir.AluOpType.add,
                        op1=mybir.AluOpType.add,
                    )
                    nc.sync.dma_start(out=out_r[:, och, tsl], in_=o_sb)

            # three-deep deferral: the ST pair for step i+3 is emitted before
            # the zq/den consumers of step i, so the PE finishes the next
            # exp's inputs well before ScalarE needs them and the stream is
            # paced purely by the exp rate (the extra depth also covers the
            # longer latency of the VectorE-offloaded exps)
            prev = []
            pend = None
            for lt in range(NLT):
                lsl = slice(lt * LT, (lt + 1) * LT)
                zq0 = ps_zq.tile([P, LT], f32, tag="zq0")
                zq1 = ps_zq.tile([P, LT], f32, tag="zq1")
                zq_new = (zq0, zq1)
                den_new = ps_small.tile([1, LT], f32, tag="den")
                dacc = dpacc.tile([P, 2, LT], f32, name="dacc", tag="dacc")
                dacc_bf = dpacc.tile(
                    [P, 2, LT], bf16, name="dacc_bf", tag="daccbf")
                dacc_started = [False]

                def emit_dacc(src_ap, eng, dacc=dacc, started=dacc_started):
                    flat = dacc.rearrange("p h l -> p (h l)")
                    if not started[0]:
                        eng.tensor_copy(out=flat, in_=src_ap)
                        started[0] = True
                    else:
                        eng.tensor_add(out=flat, in0=flat, in1=src_ap)

                dacc_pend = None
                for jp in range(JP):
                    if dacc_pend is not None:
                        # den partial for an ACT-produced step, one full
                        # period later so the exp is surely done and the
                        # DVE FIFO never blocks on it
                        emit_dacc(dacc_pend, nc.vector)
                        dacc_pend = None
                    if jp == JP - 1:
                        # dacc is complete (last contributor flushed above);
                        # cast to bf16 now so the boundary's den-reduce
                        # matmuls never wait on VectorE
                        nc.vector.tensor_copy(out=dacc_bf, in_=dacc)
                    last_split = (lt == NLT - 1 and jp == JP - 1)
                    stp = ps_st.tile([P, 2, LT], f32, tag="st")
                    for h in range(2):
                        jc = jp * 2 + h
                        nc.tensor.matmul(
                            stp[:, h, :],
                            y8_sb[:, :, jc * P:(jc + 1) * P],
                            qm_sb[:, :, lsl],
                            start=True, stop=True, perf_mode=DR,
                        )
                    # -2.5 logit shift keeps exp within fp8e4 range (max
                    # +-240); it cancels exactly in the softmax
                    if not last_split:
                        if jp in OFF_EXP:
                            isb = ipool.tile([P, 2, LT], u8, name="isb")
                            nc.vector.tensor_scalar(
                                out=isb.rearrange("p h l -> p (h l)"),
                                in0=stp.rearrange("p h l -> p (h l)"),
                                scalar1=SCHRA_A,
                                scalar2=SCHRA_B,
                                op0=mybir.AluOpType.mult,
                                op1=mybir.AluOpType.add,
                            )
                            e8 = isb.bitcast(f8)
                            if jp in OFF_DEN:
                                dacc_pend = e8.rearrange("p h l -> p (h l)")
                        else:
                            e8 = epool.tile([P, 2, LT], f8)
                            nc.scalar.activation(
                                out=e8.rearrange("p h l -> p (h l)"),
                                in_=stp.rearrange("p h l -> p (h l)"),
                                func=mybir.ActivationFunctionType.Exp,
                                scale=float(SCALE),
                                bias=shift_sb,
                            )
                            if jp in OFF_DEN:
                                dacc_pend = e8.rearrange("p h l -> p (h l)")
                        if len(prev) == 4:
                            emit_zq_den(*prev.pop(0))
                        prev.append((lt, jp, e8, zq_new, den_new, None))
                    else:
                        # very last step: process in two l-halves so the
                        # serial softmax tail operates on half tiles and
                        # pipelines across engines
                        HL = LT // 2
                        for hh in range(2):
                            hsl = slice(hh * HL, (hh + 1) * HL)
                            e8h = epool.tile([P, 2, HL], f8)
                            nc.scalar.activation(
                                out=e8h,
                                in_=stp[:, :, hsl],
                                func=mybir.ActivationFunctionType.Exp,
                                scale=float(SCALE),
                                bias=shift_sb,
                            )
                            prev.append((lt, jp, e8h, zq_new, den_new, hsl))
                        while len(prev) > 2:
                            emit_zq_den(*prev.pop(0))
                    if lt > 0 and jp == 3:
                        # previous tile's accumulators just completed (its
                        # jp=15 consumers were emitted above): fold its
                        # VectorE den partials, then stage it out
                        emit_den_reduce(p_dacc, p_dacc_bf, last_den)
                        pend_new = emit_staging(lt - 1, last_zq, last_den)
                        if pend is not None:
                            emit_tail(*pend)
                        pend = pend_new
                last_zq, last_den = zq_new, den_new
                p_dacc, p_dacc_bf = dacc, dacc_bf

            # drain: the two final half-steps, the last deferred tail, and a
            # per-half low-latency tail (PE broadcast of 1/den)
            if pend is not None:
                emit_tail(*pend)
            rbc_ps = ps_small.tile([P, LT], f32, tag="rbcps")
            rbc_sb = rpool.tile([P, LT], f32, tag="rbc")
            HL = LT // 2
            for item in prev:
                emit_zq_den(*item)
            emit_den_reduce(p_dacc, p_dacc_bf, last_den)
            for item in prev:
                hsl = item[5]
                r_sb = rpool.tile([1, HL], f32, tag="r")
                nc.vector.reciprocal_approx_fast(
                    out=r_sb, in_=last_den[:, hsl])
                r_bf = rpool.tile([1, HL], bf16, tag="rbf")
                nc.vector.tensor_copy(out=r_bf, in_=r_sb)
                nc.tensor.matmul(
                    rbc_ps[:, hsl], ones_row, r_bf, start=True, stop=True)
                nc.scalar.activation(
                    out=rbc_sb[:, hsl], in_=rbc_ps[:, hsl],
                    func=mybir.ActivationFunctionType.Copy,
                )
                base = (NLT - 1) * LT
                tslh = slice(base + hsl.start, base + hsl.stop)
                for och in range(A):
                    o_sb = opool.tile([P, HL], f32, name="o_sb", tag="osb")
                    nc.vector.tensor_mul(
                        out=o_sb, in0=last_zq[och][:, hsl],
                        in1=rbc_sb[:, hsl])
                    nc.vector.scalar_tensor_tensor(
                        out=o_sb,
                        in0=o_sb,
                        scalar=bo2_sb[:, och:och + 1],
                        in1=xres_sb[:, och, tslh],
                        op0=mybir.AluOpType.add,
                        op1=mybir.AluOpType.add,
                    )
                    # final stores issue from otherwise-idle engine queues so
                    # the ~0.6us per-issue cost doesn't serialize on Sync
                    eng = [nc.gpsimd, nc.gpsimd, nc.sync, nc.scalar][
                        (0 if hsl.start == 0 else 2) + och]
                    eng.dma_start(out=out_r[:, och, tslh], in_=o_sb)

    nc.compile()
    return nc


_NC_CACHE = {}


def _use_v2(matmul_dt_name=MATMUL_DT, fp8_level=FP8_LEVEL):
    return matmul_dt_name == "bfloat16" and fp8_level >= 3 and not USE_V1


def _get_nc(key=None):
    if key is None:
        key = (MATMUL_DT, FP8_LEVEL)
    if key not in _NC_CACHE:
        if _use_v2(*key):
            _NC_CACHE[key] = build_fp8_v2()
        else:
            _NC_CACHE[key] = build_nc(*key)
    return _NC_CACHE[key]


def _shuf(arr, dt):
    """[C, w] -> partition-major [128, A, w] (c = a*128 + p)."""
    c, w = arr.shape
    return np.ascontiguousarray(
        arr.reshape(A, P, w).transpose(1, 0, 2)).astype(dt)


def make_in_maps_v2(x, y, Wq, bq, Wk, bk, Wv, bv, Wo, bo):
    import ml_dtypes

    f32, f64 = np.float32, np.float64
    f8np = ml_dtypes.float8_e4m3
    xf = np.asarray(x, f32).reshape(B, C, N)
    yf = np.asarray(y, f32).reshape(B, C, N)
    Wq64, Wk64, Wv64, Wo64 = (np.asarray(w, f64) for w in (Wq, Wk, Wv, Wo))
    bq64, bv64, bo64 = (np.asarray(b, f64) for b in (bq, bv, bo))
    mT = _shuf(np.ascontiguousarray((Wk64.T @ Wq64).T), f8np)
    moTa = _shuf(np.ascontiguousarray((Wo64 @ Wv64).T), f8np)
    bw = (Wk64.T @ bq64).astype(f32).reshape(A, P).T.copy()
    bo2 = (bo64 + Wo64 @ bv64).astype(f32).reshape(A, P).T.copy()
    y8c = np.clip(yf, -240, 240)
    in_maps = []
    for core in range(8):
        b, h = divmod(core, 2)
        xs = np.ascontiguousarray(xf[b][:, h * NQ:(h + 1) * NQ])
        in_maps.append({
            "mx": np.concatenate([mT, _shuf(xs, f8np)], axis=2),
            "moTa": moTa, "bw": bw, "bo2": bo2,
            "y8": _shuf(y8c[b], f8np),
            "xres": _shuf(xs, f32),
        })
    return in_maps


def make_in_maps(x, y, Wq, bq, Wk, bk, Wv, bv, Wo, bo,
                 matmul_dt_name: str = MATMUL_DT, fp8_level: int = FP8_LEVEL):
    if _use_v2(matmul_dt_name, fp8_level):
        return make_in_maps_v2(x, y, Wq, bq, Wk, bk, Wv, bv, Wo, bo)
    f32 = np.float32
    f64 = np.float64
    is_bf16 = matmul_dt_name == "bfloat16"
    st_fp8 = fp8_level >= 1 and is_bf16
    proj_fp8 = fp8_level >= 2 and is_bf16
    val_fp8 = fp8_level >= 3 and is_bf16
    if is_bf16:
        import ml_dtypes

        mnp = ml_dtypes.bfloat16
        f8np = ml_dtypes.float8_e4m3
    else:
        mnp = np.float32
        f8np = None
    xnp = f8np if proj_fp8 else mnp
    xf = np.asarray(x, f32).reshape(B, C, N)
    yf = np.asarray(y, f32).reshape(B, C, N)
    Wq64, Wk64, Wv64, Wo64 = (np.asarray(w, f64) for w in (Wq, Wk, Wv, Wo))
    bq64, bv64, bo64 = (np.asarray(b, f64) for b in (bq, bv, bo))
    mT = np.ascontiguousarray((Wk64.T @ Wq64).T).astype(xnp)
    moTa = np.ascontiguousarray((Wo64 @ Wv64).T).astype(xnp)
    bw = (Wk64.T @ bq64).astype(f32)
    bo2 = (bo64 + Wo64 @ bv64).astype(f32)
    if st_fp8:
        y8 = np.clip(yf, -240, 240).astype(f8np)
    in_maps = []
    for core in range(8):
        b, h = divmod(core, 2)
        xs = np.ascontiguousarray(xf[b][:, h * NQ:(h + 1) * NQ])
        m = {
            "x": xs.astype(xnp) if xnp is not np.float32 else xs,
            "mT": mT, "moTa": moTa,
            "bw": bw, "bo2": bo2,
        }
        if not proj_fp8:
            m["y"] = yf[b].astype(mnp) if mnp is not np.float32 else yf[b]
        if st_fp8:
            m["y8"] = y8[b]
        if is_bf16:
            m["xres"] = xs
        in_maps.append(m)
    return in_maps


def kernel(x, y, Wq, bq, Wk, bk, Wv, bv, Wo, bo):
    import contextlib

    import jax

    nc = _get_nc()
    in_maps = make_in_maps(x, y, Wq, bq, Wk, bk, Wv, bv, Wo, bo)
    # Pin the axon (NeuronCore) backend: run_bass_via_pjrt uses jax.devices(),
    # which follows the ambient default platform and silently miscomputes if a
    # caller set the default to CPU.
    try:
        axon_devs = jax.devices("axon")
    except RuntimeError:
        axon_devs = None
    ctx = jax.default_device(axon_devs[0]) if axon_devs else contextlib.nullcontext()
    with ctx:
        res = bass_utils.run_bass_kernel_spmd(nc, in_maps, core_ids=list(range(8)))
    v2 = _use_v2()
    out = np.empty((B, C, N), np.float32)
    for core in range(8):
        b, h = divmod(core, 2)
        o = np.asarray(res.results[core]["out"])
        if v2:  # [P, A, NQ] -> [C, NQ]
            o = o.transpose(1, 0, 2).reshape(C, NQ)
        out[b][:, h * NQ:(h + 1) * NQ] = o
    return out.reshape(B, C, H, W)

